# revision 1
# baseline (speedup 1.0000x reference)
"""Trainium2 Bass kernel for the ConvBranch (Mamba-style) model.

Sharding: 8 cores = 4 batches x 2 DI-halves.
  core c -> batch b = c//2, half m = c%2 (owns DI channels [m*512,(m+1)*512)).
Dense matmuls (in_proj/conv/x_proj/out_proj) are replicated within a pair;
the selective-scan trio is sharded by DI-half; gated y halves are exchanged
with chunked AllGathers per layer (overlapped with the scan).

Scan layout (n-sliced): iterate (kt, n); tile [128 d-channels, TS] per state
index n. A[:, n] is a per-partition scalar so dA = exp(A_n * dt) is a plain
Activation op; B_n / C_n rows are partition-replicated once per layer via a
DRAM-staged broadcast DMA, making dBu / y-mult all-SBUF bf16 ops that split
between DVE (4x mode) and GpSimd. y accumulates over n via identity matmuls
into PSUM.

Activation tables are loaded manually (gelu / silu / ln+exp sets) to stop
the compiler's per-function table thrash.
"""

import sys

sys.path.insert(0, "/opt/trn_rl_repo")

from contextlib import ExitStack

import numpy as np
import ml_dtypes

import concourse.bass as bass
import concourse.bacc as bacc
import concourse.tile as tile
from concourse import mybir
from concourse.bass_utils import run_bass_kernel_spmd
from concourse.tile_rust import add_dep_helper

F32 = mybir.dt.float32
BF16 = mybir.dt.bfloat16
NPBF16 = ml_dtypes.bfloat16
AF = mybir.ActivationFunctionType
OP = mybir.AluOpType

B, T, F = 4, 2048, 128
DM, L, STRIDE, KF = 512, 4, 4, 2
N, DC, E = 16, 4, 2
DI = E * DM            # 1024
R = (DM + 15) // 16    # 32
K = KF * STRIDE        # 8
TS = T // STRIDE       # 512
EPS = 1e-5
DH = DI // 2           # 512 channels per core half
NC_CORES = 8
GROUPS = [[0, 1], [2, 3], [4, 5], [6, 7]]

# act_info.json set ids (gen3): 6 = ln+exp(+square/copy), 10 = gelu, 18 = silu
SET_LNEXP, SET_GELU, SET_SILU = 6, 10, 18

# scan product placement: which n-indices run on gpsimd (rest on DVE 4x STT)
POOL_DBU = frozenset(n for n in range(N) if n % 2 == 0)
POOL_TMP = frozenset(n for n in range(N) if n % 2 == 1)
CC_CHUNKS = 2          # y-exchange chunks per layer (kt tiles per chunk = 4//CC_CHUNKS)

_CACHE = {}
_DEBUG = False


def _emit(ctx, tc, ins, out, dbgs=None):
    nc = tc.nc

    def dbg(name, ap):
        if dbgs is None:
            return
        t = nc.dram_tensor("dbg_" + name, list(ap.shape), ap.dtype,
                           kind="ExternalOutput")
        nc.sync.dma_start(out=t[...], in_=ap)
        dbgs.append("dbg_" + name)

    # Manual activation-table management. The Tile scheduler reorders freely,
    # so each set-specific activation gets a no-sync edge onto the most recent
    # table load, and each load gets edges onto every tracked activation since
    # the previous load. Same-engine, ordering-only: no semaphores synthesized.
    _tbl = {"load": None, "since": []}

    def load_table(set_id):
        inst = nc.scalar.add_instruction(
            mybir.InstLoadActFuncSet(
                name=nc.get_next_instruction_name(),
                act_func_set_id=set_id, ins=[], outs=[]))
        for p in _tbl["since"]:
            add_dep_helper(inst.ins, p, sync=False, reason="act-table order")
        if _tbl["load"] is not None:
            add_dep_helper(inst.ins, _tbl["load"], sync=False,
                           reason="act-table order")
        _tbl["since"] = []
        _tbl["load"] = inst.ins

    def tact(res):
        """Track a set-specific activation: pin it after the current load."""
        if _tbl["load"] is not None:
            add_dep_helper(res.ins, _tbl["load"], sync=False,
                           reason="act-table order")
        _tbl["since"].append(res.ins)
        return res

    consts = ctx.enter_context(tc.tile_pool(name="consts", bufs=1))
    wpool = ctx.enter_context(tc.tile_pool(name="wpool", bufs=2))
    work = ctx.enter_context(tc.tile_pool(name="work", bufs=1))
    scanp = ctx.enter_context(tc.tile_pool(name="scanp", bufs=3))
    pA = ctx.enter_context(tc.tile_pool(name="pA", bufs=3, space="PSUM"))
    pB = ctx.enter_context(tc.tile_pool(name="pB", bufs=2, space="PSUM"))
    py = ctx.enter_context(tc.tile_pool(name="py", bufs=2, space="PSUM"))
    dram = ctx.enter_context(tc.tile_pool(name="dram", bufs=1, space="DRAM"))

    # ---- persistent constants ----
    xt_sb = wpool.tile([128, T + K - 1], BF16, name="wotr")
    nc.sync.dma_start(out=xt_sb, in_=ins["xt"][:, :])
    w1t_sb = wpool.tile([128, K, DM], BF16, name="wint")
    nc.sync.dma_start(out=w1t_sb, in_=ins["w1t"].rearrange("k f m -> f k m"))
    cb_sb = consts.tile([128, 4], F32)
    nc.sync.dma_start(out=cb_sb, in_=ins["cb"][:, :])
    ident_sb = consts.tile([128, 128], F32)
    nc.sync.dma_start(out=ident_sb, in_=ins["ident"][:, :])
    identb_sb = consts.tile([128, 128], BF16)
    nc.sync.dma_start(out=identb_sb, in_=ins["identb"][:, :])
    nwrow_sb = consts.tile([1, DM], BF16)
    nc.sync.dma_start(out=nwrow_sb, in_=ins["nwrow"][:, :])
    nbc_sb = consts.tile([128, 4], F32)
    nc.sync.dma_start(out=nbc_sb, in_=ins["nbc"][:, :])
    ones128_sb = consts.tile([128, 1], BF16)
    nc.vector.memset(ones128_sb, 1.0)
    eps1_sb = consts.tile([1, 1], F32)
    nc.vector.memset(eps1_sb, EPS)
    onesf_sb = consts.tile([128, 1], F32)
    nc.vector.memset(onesf_sb, 1.0)

    # residual stream h: 4 persistent fp32 tiles [128, TS]
    h = [consts.tile([128, TS], F32, name=f"h{kt}") for kt in range(4)]

    # ---- front conv + gelu ----
    load_table(SET_GELU)
    for mt in range(4):
        ps = pA.tile([128, TS], F32, name="ps")
        for k in range(K):
            nc.tensor.matmul(
                ps,
                w1t_sb[:, k, mt * 128:(mt + 1) * 128],
                xt_sb[:, k:k + T:STRIDE],
                start=(k == 0),
                stop=(k == K - 1),
            )
        tact(nc.scalar.activation(h[mt], ps, AF.Gelu, bias=cb_sb[:, mt:mt + 1]))
    load_table(SET_LNEXP)

    def load_weights(l):
        w = {}

        def wt(name, shape, dt, src):
            t = wpool.tile(shape, dt, name=name)
            nc.sync.dma_start(out=t, in_=src)
            w[name] = t

        wt("wint", [128, 4, DI + DH], BF16,
           ins["wint"][l].rearrange("(kt p) e -> p kt e", p=128))
        wt("wotr", [128, 8, DM], BF16,
           ins["wotr"][l].rearrange("(kd p) o -> p kd o", p=128))
        wt("xpt", [128, 8, R + 2 * N], BF16,
           ins["xpt"][l].rearrange("(kd p) e -> p kd e", p=128))
        wt("dtpt", [32, DH], BF16, ins["dtpt"][l])
        wt("cwd", [128, 8, DC, 128], BF16,
           ins["cwdiag"][l].rearrange("e k p q -> p e k q"))
        wt("cb1", [128, 8], F32, ins["cb1d"][l])
        wt("dtpb", [128, 4], F32, ins["dtpb"][l])
        wt("asc", [128, 64], F32, ins["asc"][l])
        wt("dsc", [128, 4], F32, ins["dsc"][l])
        wt("lnrow", [1, DM], BF16, ins["lnrow"][l])
        wt("lnbc", [128, 4], F32, ins["lnbc"][l])
        return w

    def layernorm(lnrow, lnbc, out_dtype, name):
        """LN over the feature (partition) dim of h; affine folded into the
        rank-1 PE broadcasts: hn = (h*A + lb) - Bc, A = lw*rstd, Bc = lw*c."""
        st_m = pA.tile([128, TS], F32, name="ps")
        st_q = pA.tile([128, TS], F32, name="ps")
        for kt in range(4):
            hbt = work.tile([128, TS], BF16, name=f"hb{kt % 2}")
            nc.scalar.copy(hbt, h[kt])
            sq = work.tile([128, TS], BF16, name="sq")
            nc.scalar.activation(sq, h[kt], AF.Square)
            nc.tensor.matmul(st_m[0:1, :], ones128_sb, hbt,
                             start=(kt == 0), stop=(kt == 3))
            nc.tensor.matmul(st_q[0:1, :], ones128_sb, sq,
                             start=(kt == 0), stop=(kt == 3))
        ms = work.tile([1, 2 * TS], F32, name="ms")
        nc.vector.tensor_scalar(ms[:, 0:TS], st_m[0:1, :], 1.0 / DM, None, OP.mult)
        mu2 = work.tile([1, TS], F32, name="mu2")
        nc.scalar.activation(mu2, ms[:, 0:TS], AF.Square)
        var = work.tile([1, TS], F32, name="var")
        nc.vector.scalar_tensor_tensor(var, st_q[0:1, :], 1.0 / DM, mu2,
                                       OP.mult, OP.subtract)
        # rstd = exp(-0.5 * ln(var + eps)); c = mu * rstd   (packed [1, 2*TS])
        rc = work.tile([1, 2 * TS], F32, name="rc")
        lnv = work.tile([1, TS], F32, name="lnv")
        tact(nc.scalar.activation(lnv, var, AF.Ln, bias=eps1_sb[:, 0:1]))
        tact(nc.scalar.activation(rc[:, 0:TS], lnv, AF.Exp, scale=-0.5))
        nc.vector.tensor_tensor(out=rc[:, TS:2 * TS], in0=ms[:, 0:TS],
                                in1=rc[:, 0:TS], op=OP.mult)
        rcb = work.tile([1, 2 * TS], BF16, name="rcb")
        nc.scalar.copy(rcb, rc)
        outs = []
        for kt in range(4):
            sl = slice(kt * 128, (kt + 1) * 128)
            pa = pB.tile([128, TS], F32, name="pln")
            nc.tensor.matmul(pa, lnrow[0:1, sl], rcb[:, 0:TS],
                             start=True, stop=True)
            pbc = pB.tile([128, TS], F32, name="pln")
            nc.tensor.matmul(pbc, lnrow[0:1, sl], rcb[:, TS:2 * TS],
                             start=True, stop=True)
            t1 = work.tile([128, TS], F32, name="lnt1")
            nc.vector.tensor_tensor(out=t1, in0=h[kt], in1=pa, op=OP.mult)
            o = work.tile([128, TS], out_dtype, name=f"{name}{kt}")
            nc.vector.scalar_tensor_tensor(o, t1, lnbc[:, kt:kt + 1], pbc,
                                           OP.add, OP.subtract)
            outs.append(o)
        return outs

    # zero the causal pads of the conv input tiles once; the per-layer copy
    # only writes cols [DC-1:).
    xi_pad = [work.tile([128, DC - 1 + TS], BF16, name=f"xipad{et}")
              for et in range(8)]
    for et in range(8):
        nc.vector.memset(xi_pad[et][:, 0:DC - 1], 0.0)

    wcur = load_weights(0)
    for l in range(L):
        wnext = load_weights(l + 1) if l + 1 < L else None
        wint_sb, wotr_sb, xpt_sb = wcur["wint"], wcur["wotr"], wcur["xpt"]
        dtpt_sb, cwd_sb, cb1_sb = wcur["dtpt"], wcur["cwd"], wcur["cb1"]
        dtpb_sb, asc_sb, dsc_sb = wcur["dtpb"], wcur["asc"], wcur["dsc"]

        # ---- LN ----
        hn = layernorm(wcur["lnrow"], wcur["lnbc"], BF16, "hn")
        if l == 0:
            dbg("hn0", hn[0])

        # ---- in_proj xi tiles 0..7 (padded for conv) ----
        for et in range(8):
            ps = pA.tile([128, TS], F32, name="ps")
            for kt in range(4):
                nc.tensor.matmul(ps, wint_sb[:, kt, et * 128:(et + 1) * 128],
                                 hn[kt], start=(kt == 0), stop=(kt == 3))
            nc.scalar.copy(xi_pad[et][:, DC - 1:DC - 1 + TS], ps)

        # ---- causal depthwise conv1d + silu ----
        load_table(SET_SILU)
        xi = []
        for et in range(8):
            psc = pA.tile([128, TS], F32, name="ps")
            for k in range(DC):
                nc.tensor.matmul(psc, cwd_sb[:, et, k, :],
                                 xi_pad[et][:, k:k + TS],
                                 start=(k == 0), stop=(k == DC - 1))
            xit = work.tile([128, TS], BF16, name=f"xi{et}")
            tact(nc.scalar.activation(xit, psc, AF.Silu, bias=cb1_sb[:, et:et + 1]))
            xi.append(xit)
        if l == 0:
            dbg("xi0", xi[0])

        # ---- x_proj -> x_dbl [64, TS]; rows 0:32 dt_raw, 32:48 B, 48:64 C ----
        psx = pA.tile([64, TS], F32, name="ps")
        for et in range(8):
            nc.tensor.matmul(psx, xpt_sb[:, et, :], xi[et],
                             start=(et == 0), stop=(et == 7))
        xdbl = work.tile([64, TS], BF16, name="xdbl")
        nc.scalar.copy(xdbl, psx)
        if l == 0:
            dbg("xdbl", xdbl)

        # ---- stage B/C rows to DRAM, broadcast to all partitions ----
        # bcrep[:, n, :] = B_n;  bcrep[:, 16+n, :] = C_n
        bcst = dram.tile([2 * N, TS], BF16, name="bcst")
        nc.sync.dma_start(out=bcst, in_=xdbl[R:R + 2 * N, :])
        bcrep = work.tile([128, 2 * N, TS], BF16, name="bcrep")
        for q in range(4):
            for half in range(2):
                r0 = half * N + 4 * q
                nc.sync.dma_start(
                    out=bcrep[:, r0:r0 + 4, :],
                    in_=bcst[r0:r0 + 4, :].unsqueeze(0).to_broadcast(
                        [128, 4, TS]))
        if l == 0:
            dbg("brep", bcrep[:, 0, :])
            dbg("crep", bcrep[:, N, :])

        # ---- dt: softplus(dt_proj @ dt_raw + b) = ln(1 + exp(.)) ----
        load_table(SET_LNEXP)
        w = []
        dtu = []
        for kt in range(4):
            psd = pA.tile([128, TS], F32, name="ps")
            nc.tensor.matmul(psd, dtpt_sb[:, kt * 128:(kt + 1) * 128],
                             xdbl[0:32, :], start=True, stop=True)
            edt = work.tile([128, TS], F32, name="edt")
            tact(nc.scalar.activation(edt, psd, AF.Exp, bias=dtpb_sb[:, kt:kt + 1]))
            wt = work.tile([128, TS], BF16, name=f"w{kt}")
            tact(nc.scalar.activation(wt, edt, AF.Ln, bias=onesf_sb[:, 0:1]))
            w.append(wt)
            du = work.tile([128, TS], BF16, name=f"dtu{kt}")
            nc.vector.tensor_tensor(out=du, in0=wt, in1=xi[kt], op=OP.mult)
            dtu.append(du)
        if l == 0:
            dbg("w0", w[0])
            dbg("dtu0", dtu[0])

        # ---- in_proj z tiles + silu (off the scan critical path) ----
        load_table(SET_SILU)
        zs = []
        for zt in range(4):
            et = 8 + zt
            ps = pA.tile([128, TS], F32, name="ps")
            for kt in range(4):
                nc.tensor.matmul(ps, wint_sb[:, kt, et * 128:(et + 1) * 128],
                                 hn[kt], start=(kt == 0), stop=(kt == 3))
            z = work.tile([128, TS], BF16, name=f"zs{zt}")
            tact(nc.scalar.activation(z, ps, AF.Silu))
            zs.append(z)
        if l == 0:
            dbg("zs0", zs[0])
        load_table(SET_LNEXP)

        # ---- scan (n-sliced) + gating + chunked y-exchange ----
        KT_PER_CC = 4 // CC_CHUNKS
        pso = None
        for kt in range(4):
            pyt = py.tile([128, TS], F32, name="py")
            for n in range(N):
                col = kt * 16 + n
                dA = scanp.tile([128, TS], BF16, name="dA")
                tact(nc.scalar.activation(dA, w[kt], AF.Exp,
                                          scale=asc_sb[:, col:col + 1]))
                dBu = scanp.tile([128, TS], BF16, name="dBu")
                if n in POOL_DBU:
                    nc.gpsimd.tensor_tensor(out=dBu, in0=dtu[kt],
                                            in1=bcrep[:, n, :], op=OP.mult)
                else:
                    nc.vector.tensor_tensor(out=dBu, in0=dtu[kt],
                                            in1=bcrep[:, n, :], op=OP.mult)
                hs = scanp.tile([128, TS], BF16, name="hs")
                nc.vector.tensor_tensor_scan(hs, dA, dBu, 0.0, OP.mult, OP.add)
                tmp = scanp.tile([128, TS], BF16, name="tmp")
                if n in POOL_TMP:
                    nc.gpsimd.tensor_tensor(out=tmp, in0=hs,
                                            in1=bcrep[:, N + n, :], op=OP.mult)
                else:
                    nc.vector.tensor_tensor(out=tmp, in0=hs,
                                            in1=bcrep[:, N + n, :], op=OP.mult)
                nc.tensor.matmul(pyt, identb_sb, tmp,
                                 start=(n == 0), stop=(n == N - 1))
                if l == 0 and kt == 0 and n == 0:
                    dbg("dA00", dA)
                    dbg("dBu00", dBu)
                    dbg("hs00", hs)
            # gating: yg = (y + xi * D) * silu(z)  (z pre-silu'd in zs)
            c = kt // KT_PER_CC
            ki = kt % KT_PER_CC
            if ki == 0:
                ygp = work.tile([128, KT_PER_CC, TS], BF16, name=f"ygp{c}")
            g1 = work.tile([128, TS], BF16, name="g1")
            nc.vector.scalar_tensor_tensor(g1, xi[kt], dsc_sb[:, kt:kt + 1],
                                           pyt, OP.mult, OP.add)
            nc.vector.tensor_tensor(out=ygp[:, ki, :], in0=g1, in1=zs[kt],
                                    op=OP.mult)
            if l == 0 and kt == 0:
                dbg("g10", g1)

            if ki == KT_PER_CC - 1:
                # exchange this chunk of gated y
                ccin = dram.tile([KT_PER_CC * 128, TS], BF16, name=f"ccin{c}")
                nc.sync.dma_start(
                    out=ccin.rearrange("(k p) t -> p k t", p=128), in_=ygp)
                ccout = dram.tile([KT_PER_CC * 256, TS], BF16, name=f"ccout{c}")
                nc.gpsimd.collective_compute(
                    "AllGather", OP.bypass, replica_groups=GROUPS,
                    ins=[ccin[:, :]], outs=[ccout[:, :]],
                )
                ygf = work.tile([128, 2 * KT_PER_CC, TS], BF16, name=f"ygf{c}")
                nc.sync.dma_start(
                    out=ygf, in_=ccout.rearrange("(kd p) t -> p kd t", p=128))
                # out_proj partial accumulation for this chunk
                if pso is None:
                    pso = [pA.tile([128, TS], F32, name="ps"),
                           pA.tile([128, TS], F32, name="ps"),
                           pB.tile([128, TS], F32, name="pln"),
                           pB.tile([128, TS], F32, name="pln")]
                nkd = 2 * KT_PER_CC
                for mt in range(4):
                    for kd in range(nkd):
                        nc.tensor.matmul(
                            pso[mt],
                            wotr_sb[:, c * nkd + kd, mt * 128:(mt + 1) * 128],
                            ygf[:, kd, :],
                            start=(c == 0 and kd == 0),
                            stop=(c == CC_CHUNKS - 1 and kd == nkd - 1))

        # ---- residual ----
        for mt in range(4):
            nc.vector.tensor_tensor(out=h[mt], in0=h[mt], in1=pso[mt], op=OP.add)
        if l == 0:
            dbg("hl0", h[0])

        wcur = wnext

    # ---- final LN ----
    hnf = layernorm(nwrow_sb, nbc_sb, F32, "hnf")

    # ---- transpose + repeat-interleave upsample + store ----
    for ct in range(4):
        hT = work.tile([128, DM], F32, name=f"hT{ct}")
        for kt in range(4):
            pt = pA.tile([128, 128], F32, name="ps")
            nc.tensor.transpose(pt, hnf[kt][:, ct * 128:(ct + 1) * 128], ident_sb)
            nc.vector.tensor_copy(out=hT[:, kt * 128:(kt + 1) * 128], in_=pt)
        for j in range(STRIDE):
            base = 512 * ct + j
            nc.sync.dma_start(out=out[base:base + 509:STRIDE, :], in_=hT)


def _build_nc():
    nc = bacc.Bacc("TRN2", num_devices=NC_CORES)
    ins = {}

    def din(name, shape, dt):
        ins[name] = nc.dram_tensor(name, list(shape), dt, kind="ExternalInput")

    din("xt", (128, T + K - 1), BF16)
    din("w1t", (K, 128, DM), BF16)
    din("cb", (128, 4), F32)
    din("ident", (128, 128), F32)
    din("identb", (128, 128), BF16)
    din("nwrow", (1, DM), BF16)
    din("nbc", (128, 4), F32)
    din("wint", (L, DM, DI + DH), BF16)
    din("wotr", (L, DI, DM), BF16)
    din("xpt", (L, DI, R + 2 * N), BF16)
    din("dtpt", (L, R, DH), BF16)
    din("cwdiag", (L, 8, DC, 128, 128), BF16)
    din("cb1d", (L, 128, 8), F32)
    din("dtpb", (L, 128, 4), F32)
    din("asc", (L, 128, 64), F32)
    din("dsc", (L, 128, 4), F32)
    din("lnrow", (L, 1, DM), BF16)
    din("lnbc", (L, 128, 4), F32)
    out = nc.dram_tensor("out", [T, DM], F32, kind="ExternalOutput")

    dbgs = [] if _DEBUG else None
    with ExitStack() as ctx:
        tc = ctx.enter_context(tile.TileContext(nc))
        _emit(ctx, tc, ins, out, dbgs)
    nc.compile()
    _CACHE["dbgs"] = dbgs
    return nc


def _prep_core_inputs(c, inputs):
    b, m = c // 2, c % 2
    bf = lambda a: np.ascontiguousarray(a).astype(NPBF16)
    f32 = lambda a: np.ascontiguousarray(a).astype(np.float32)

    x = np.asarray(inputs["x"], np.float32)
    xt = np.zeros((128, T + K - 1), np.float32)
    xt[:, K - 1:] = x[b].T
    w1t = np.asarray(inputs["conv_w"], np.float32).transpose(2, 1, 0)  # [K,F,DM]
    cb = np.asarray(inputs["conv_b"], np.float32).reshape(4, 128).T
    ident = np.eye(128, dtype=np.float32)
    nwrow = np.asarray(inputs["norm_w"], np.float32).reshape(1, DM)
    nbc = np.asarray(inputs["norm_b"], np.float32).reshape(4, 128).T

    # per-core DI channel permutation: own half first
    own = np.arange(m * DH, (m + 1) * DH)
    oth = np.arange((1 - m) * DH, (2 - m) * DH)
    perm = np.concatenate([own, oth])

    in_w = np.asarray(inputs["in_proj_w"], np.float32)    # [L, 2*DI, DM]
    wint = np.empty((L, DM, DI + DH), np.float32)
    for l in range(L):
        wtp = in_w[l].T                                   # [DM, 2*DI]
        wint[l, :, :DI] = wtp[:, perm]                    # xi, permuted
        wint[l, :, DI:] = wtp[:, DI + own]                # z own half
    # out_proj rows in chunk-arrival order (unpermuted channels):
    # chunk c: [h0 ch 256c..256c+256, h1 ch 512+256c..512+256c+256]
    wot = np.asarray(inputs["out_proj_w"], np.float32).transpose(0, 2, 1)  # [L,DI,DM]
    KT_PER_CC = 4 // CC_CHUNKS
    row_order = []
    for cc in range(CC_CHUNKS):
        w0 = cc * KT_PER_CC * 128
        row_order.extend(range(w0, w0 + KT_PER_CC * 128))
        row_order.extend(range(DH + w0, DH + w0 + KT_PER_CC * 128))
    wotr = wot[:, row_order, :]
    xpt = np.asarray(inputs["x_proj_w"], np.float32).transpose(0, 2, 1)[:, perm, :]
    dtpt = np.asarray(inputs["dt_proj_w"], np.float32).transpose(0, 2, 1)[:, :, own]
    cw1d = np.asarray(inputs["conv1d_w"], np.float32)[:, perm, :]
    cwdiag = np.zeros((L, 8, DC, 128, 128), np.float32)
    ii = np.arange(128)
    for l in range(L):
        for et in range(8):
            for k in range(DC):
                cwdiag[l, et, k, ii, ii] = cw1d[l, et * 128:(et + 1) * 128, k]
    cb1d = np.asarray(inputs["conv1d_b"], np.float32)[:, perm].reshape(L, 8, 128)
    cb1d = cb1d.transpose(0, 2, 1)
    dtpb = np.asarray(inputs["dt_proj_b"], np.float32)[:, own].reshape(L, 4, 128)
    dtpb = dtpb.transpose(0, 2, 1)
    A = -np.exp(np.asarray(inputs["A_log"], np.float32))[:, own, :]  # [L, DH, N]
    # asc[l, p, 16*kt + n] = A[l, kt*128 + p, n]
    asc = A.reshape(L, 4, 128, N).transpose(0, 2, 1, 3).reshape(L, 128, 64)
    dsc = np.asarray(inputs["D_skip"], np.float32)[:, own].reshape(L, 4, 128)
    dsc = dsc.transpose(0, 2, 1)
    lnrow = np.asarray(inputs["ln_w"], np.float32).reshape(L, 1, DM)
    lnbc = np.asarray(inputs["ln_b"], np.float32).reshape(L, 4, 128)
    lnbc = lnbc.transpose(0, 2, 1)

    return dict(
        xt=bf(xt), w1t=bf(w1t), cb=f32(cb), ident=ident,
        identb=bf(np.eye(128, dtype=np.float32)), nwrow=bf(nwrow),
        nbc=f32(nbc),
        wint=bf(wint), wotr=bf(wotr), xpt=bf(xpt), dtpt=bf(dtpt),
        cwdiag=bf(cwdiag), cb1d=f32(cb1d), dtpb=f32(dtpb), asc=f32(asc),
        dsc=f32(dsc), lnrow=bf(lnrow), lnbc=f32(lnbc),
    )


def kernel(trace=False, **inputs):
    if "nc" not in _CACHE:
        _CACHE["nc"] = _build_nc()
    nc = _CACHE["nc"]
    in_maps = [_prep_core_inputs(c, inputs) for c in range(NC_CORES)]
    res = run_bass_kernel_spmd(nc, in_maps, list(range(NC_CORES)), trace=trace)
    out = np.stack([np.asarray(res.results[2 * b]["out"], np.float32)
                    for b in range(B)])
    _CACHE["last_result"] = res
    return out



# revision 45
# speedup vs baseline: 146.3890x; 146.3890x over previous
"""Trainium2 Bass kernel for the ConvBranch (Mamba-style) model.

Sharding: 8 cores = 4 batches x 2 DI-halves.
  core c -> batch b = c//2, half m = c%2 (owns DI channels [m*512,(m+1)*512)).
Dense matmuls (in_proj/conv/x_proj/out_proj) are replicated within a pair;
the selective-scan trio is sharded by DI-half; gated y halves are exchanged
with chunked AllGathers per layer (overlapped with the scan).

Scan layout (n-quad fused): per (kt, q) process 4 state indices in one
[128, 4, TS+1] tile; a zero separator column between n-segments resets the
scan state, so one tensor_tensor_scan covers 4 independent recurrences.
dBu / tmp are single quad TT ops with the B/C rows broadcast via stride-0
access patterns. Scans and tmp run on DVE (HW GPSIMD cannot execute
TensorScalarPtr or touch PSUM); most dBu quads run on GpSimd, produced
ahead of the scan chain. dA exps on Act; xi*D folds into the y-PSUM via a
diagonal matmul.

The gated-y exchange is fp8(e4m3): two AllGathers per layer ([kt0,kt1],
[kt2,kt3]), the first hidden behind the second half of the scan. out_proj
consumes the fp8 payload directly with DoubleRow fp8 matmuls (fp8 wotr).

Activation tables are loaded manually; two set switches per layer
(SILU block, then LNEXP block).

Output is [TS, DM] per core; the x4 repeat-interleave upsample happens on
the host.
"""

import sys

sys.path.insert(0, "/opt/trn_rl_repo")

from contextlib import ExitStack

import numpy as np
import ml_dtypes

import concourse.bass as bass
import concourse.bacc as bacc
import concourse.tile as tile
from concourse import mybir
from concourse.bass_utils import run_bass_kernel_spmd
from concourse.tile_rust import add_dep_helper

F32 = mybir.dt.float32
BF16 = mybir.dt.bfloat16
FP8 = mybir.dt.float8e4
NPBF16 = ml_dtypes.bfloat16
AF = mybir.ActivationFunctionType
OP = mybir.AluOpType

B, T, F = 4, 2048, 128
DM, L, STRIDE, KF = 512, 4, 4, 2
N, DC, E = 16, 4, 2
DI = E * DM            # 1024
R = (DM + 15) // 16    # 32
K = KF * STRIDE        # 8
TS = T // STRIDE       # 512
TSP = TS + 1           # +1 separator col for n-fused scans
EPS = 1e-5
DH = DI // 2           # 512 channels per core half
NC_CORES = 8
GROUPS = [[0, 1], [2, 3], [4, 5], [6, 7]]
NQ = 4                 # n-quad group size

# act_info.json set ids (gen3): 6 = ln+exp(+square/copy), 10 = gelu, 18 = silu
SET_LNEXP, SET_GELU, SET_SILU = 6, 10, 18

# dBu quad-TT placement (Pool cannot run scans or stt on HW; TT runs at
# 0.42 efficiency there). dBu only needs dtu+bcrep so Pool can produce it
# ahead of the scan chain; tmp stays on DVE (it feeds PE directly).
CC_CHUNKS = 2          # y-exchange chunks per layer (kt tiles per chunk = 2)

_CACHE = {}
_DEBUG = False


def _emit(ctx, tc, ins, out, dbgs=None):
    nc = tc.nc

    def dbg(name, ap):
        if dbgs is None:
            return
        t = nc.dram_tensor("dbg_" + name, list(ap.shape), ap.dtype,
                           kind="ExternalOutput")
        nc.sync.dma_start(out=t[...], in_=ap)
        dbgs.append("dbg_" + name)

    # Manual activation-table management (same-engine ordering-only edges).
    _tbl = {"load": None, "since": []}

    def load_table(set_id):
        inst = nc.scalar.add_instruction(
            mybir.InstLoadActFuncSet(
                name=nc.get_next_instruction_name(),
                act_func_set_id=set_id, ins=[], outs=[]))
        for p in _tbl["since"]:
            add_dep_helper(inst.ins, p, sync=False, reason="act-table order")
        if _tbl["load"] is not None:
            add_dep_helper(inst.ins, _tbl["load"], sync=False,
                           reason="act-table order")
        _tbl["since"] = []
        _tbl["load"] = inst.ins

    def tact(res):
        if _tbl["load"] is not None:
            add_dep_helper(res.ins, _tbl["load"], sync=False,
                           reason="act-table order")
        _tbl["since"].append(res.ins)
        return res

    consts = ctx.enter_context(tc.tile_pool(name="consts", bufs=1))
    wpool1 = ctx.enter_context(tc.tile_pool(name="wpool1", bufs=1))
    work = ctx.enter_context(tc.tile_pool(name="work", bufs=1))
    scana = ctx.enter_context(tc.tile_pool(name="scana", bufs=3))
    scanh = ctx.enter_context(tc.tile_pool(name="scanh", bufs=3))
    scanb = ctx.enter_context(tc.tile_pool(name="scanb", bufs=4))
    scant = ctx.enter_context(tc.tile_pool(name="scant", bufs=2))
    pA = ctx.enter_context(tc.tile_pool(name="pA", bufs=4, space="PSUM"))
    pB = ctx.enter_context(tc.tile_pool(name="pB", bufs=2, space="PSUM"))
    py = ctx.enter_context(tc.tile_pool(name="py", bufs=2, space="PSUM"))
    dram = ctx.enter_context(tc.tile_pool(name="dram", bufs=1, space="DRAM"))

    # ---- persistent constants ----
    xt_sb = wpool1.tile([128, T + K - 1], BF16, name="wotr")
    nc.sync.dma_start(out=xt_sb, in_=ins["xt"][:, :])
    w1t_sb = wpool1.tile([128, K, DM], BF16, name="cwd")
    nc.sync.dma_start(out=w1t_sb, in_=ins["w1t"].rearrange("k f m -> f k m"))
    cb_sb = consts.tile([128, 4], F32)
    nc.sync.dma_start(out=cb_sb, in_=ins["cb"][:, :])
    ident_sb = consts.tile([128, 128], F32)
    nc.sync.dma_start(out=ident_sb, in_=ins["ident"][:, :])
    identb_sb = consts.tile([128, 128], BF16)
    nc.sync.dma_start(out=identb_sb, in_=ins["identb"][:, :])
    nwrow_sb = consts.tile([1, DM], BF16)
    nc.sync.dma_start(out=nwrow_sb, in_=ins["nwrow"][:, :])
    nbc_sb = consts.tile([128, 4], F32)
    nc.sync.dma_start(out=nbc_sb, in_=ins["nbc"][:, :])
    ones128_sb = consts.tile([128, 1], BF16)
    nc.vector.memset(ones128_sb, 1.0)
    eps1_sb = consts.tile([1, 1], F32)
    nc.vector.memset(eps1_sb, EPS)
    onesf_sb = consts.tile([128, 1], F32)
    nc.vector.memset(onesf_sb, 1.0)

    # B/C broadcast target: [128, 2N, TSP]; separator cols zeroed once.
    bcrep = consts.tile([128, 2 * N, TSP], BF16, name="bcrep")
    nc.vector.memset(bcrep[:, :, TS:TSP], 0.0)

    # residual stream h: 4 persistent fp32 tiles [128, TS]
    h = [consts.tile([128, TS], F32, name=f"h{kt}") for kt in range(4)]

    # ---- front conv + gelu ----
    load_table(SET_GELU)
    for mt in range(4):
        ps = pA.tile([128, TS], F32, name="ps")
        for k in range(K):
            nc.tensor.matmul(
                ps,
                w1t_sb[:, k, mt * 128:(mt + 1) * 128],
                xt_sb[:, k:k + T:STRIDE],
                start=(k == 0),
                stop=(k == K - 1),
            )
        tact(nc.scalar.activation(h[mt], ps, AF.Gelu, bias=cb_sb[:, mt:mt + 1]))
    load_table(SET_LNEXP)

    def load_weights(l):
        w = {}

        def wt(name, shape, dt, src, pool=wpool1):
            t = pool.tile(shape, dt, name=name)
            nc.sync.dma_start(out=t, in_=src)
            w[name] = t

        wt("wint", [128, 4, DI + DH], BF16,
           ins["wint"][l].rearrange("(kt p) e -> p kt e", p=128))
        wt("wotr", [128, 8, DM], FP8,
           ins["wotr"][l].rearrange("(kd p) o -> p kd o", p=128))
        wt("xpt", [128, 8, R + 2 * N], BF16,
           ins["xpt"][l].rearrange("(kd p) e -> p kd e", p=128))
        wt("dtpt", [32, DH], BF16, ins["dtpt"][l])
        wt("cwd", [128, 8, DC, 128], BF16,
           ins["cwdiag"][l].rearrange("e k p q -> p e k q"))
        wt("cb1", [128, 8], F32, ins["cb1d"][l])
        wt("dtpb", [128, 4], F32, ins["dtpb"][l])
        wt("asc", [128, 64], F32, ins["asc"][l])
        wt("dscd", [128, 4, 128], BF16,
           ins["dscdiag"][l].rearrange("k p q -> p k q"))
        wt("lnrow", [1, DM], BF16, ins["lnrow"][l])
        wt("lnbc", [128, 4], F32, ins["lnbc"][l])
        return w

    def layernorm(lnrow, lnbc, out_dtype, name, outq=None):
        """LN over the feature (partition) dim of h; affine folded into the
        rank-1 PE broadcasts: hn = (h*A + lb) - Bc, A = lw*rstd, Bc = lw*c.
        If outq is given, write kt slices into outq[:, kt, :]."""
        st_m = pA.tile([128, TS], F32, name="ps")
        st_q = pA.tile([128, TS], F32, name="ps")
        for kt in range(4):
            hbt = work.tile([128, TS], BF16, name=f"hb{kt % 2}")
            nc.gpsimd.tensor_copy(out=hbt, in_=h[kt])
            sq = work.tile([128, TS], BF16, name="sq")
            nc.scalar.activation(sq, h[kt], AF.Square)
            nc.tensor.matmul(st_m[0:1, :], ones128_sb, hbt,
                             start=(kt == 0), stop=(kt == 3))
            nc.tensor.matmul(st_q[0:1, :], ones128_sb, sq,
                             start=(kt == 0), stop=(kt == 3))
        ms = work.tile([1, 2 * TS], F32, name="ms")
        nc.vector.tensor_scalar(ms[:, 0:TS], st_m[0:1, :], 1.0 / DM, None, OP.mult)
        mu2 = work.tile([1, TS], F32, name="mu2")
        nc.vector.tensor_tensor(out=mu2, in0=ms[:, 0:TS], in1=ms[:, 0:TS],
                                op=OP.mult)
        var = work.tile([1, TS], F32, name="var")
        nc.vector.scalar_tensor_tensor(var, st_q[0:1, :], 1.0 / DM, mu2,
                                       OP.mult, OP.subtract)
        # rstd = exp(-0.5 * ln(var + eps)); c = mu * rstd   (packed [1, 2*TS])
        rc = work.tile([1, 2 * TS], F32, name="rc")
        lnv = work.tile([1, TS], F32, name="lnv")
        tact(nc.scalar.activation(lnv, var, AF.Ln, bias=eps1_sb[:, 0:1]))
        tact(nc.scalar.activation(rc[:, 0:TS], lnv, AF.Exp, scale=-0.5))
        nc.vector.tensor_tensor(out=rc[:, TS:2 * TS], in0=ms[:, 0:TS],
                                in1=rc[:, 0:TS], op=OP.mult)
        rcb = work.tile([1, 2 * TS], BF16, name="rcb")
        nc.scalar.copy(rcb, rc)
        outs = []
        for kt in range(4):
            sl = slice(kt * 128, (kt + 1) * 128)
            pa = pB.tile([128, TS], F32, name="pln")
            nc.tensor.matmul(pa, lnrow[0:1, sl], rcb[:, 0:TS],
                             start=True, stop=True)
            pbc = pB.tile([128, TS], F32, name="pln")
            nc.tensor.matmul(pbc, lnrow[0:1, sl], rcb[:, TS:2 * TS],
                             start=True, stop=True)
            t1 = work.tile([128, TS], F32, name="lnt1")
            nc.vector.tensor_tensor(out=t1, in0=h[kt], in1=pa, op=OP.mult)
            o = outq[:, kt, :] if outq is not None else None
            if o is None:
                ot = work.tile([128, TS], out_dtype, name=f"{name}{kt}")
                o = ot
            nc.vector.scalar_tensor_tensor(o, t1, lnbc[:, kt:kt + 1], pbc,
                                           OP.add, OP.subtract)
            outs.append(o)
        return outs

    # zero the causal pads of the conv input tiles once; the per-layer copy
    # only writes cols [DC-1:).
    xi_pad = [work.tile([128, DC - 1 + TS], BF16, name=f"xipad{et}")
              for et in range(8)]
    for et in range(8):
        nc.vector.memset(xi_pad[et][:, 0:DC - 1], 0.0)

    # persistent quad tiles
    hnq = consts.tile([128, 4, TS], BF16, name="hnq")
    xiq = consts.tile([128, 4, TS], BF16, name="xiq")   # silu(conv), own half
    zq = consts.tile([128, 4, TS], BF16, name="zq")     # silu(z)
    wq = consts.tile([128, 4, TSP], BF16, name="wq")    # softplus dt
    dtuq = consts.tile([128, 4, TSP], BF16, name="dtuq")
    nc.vector.memset(wq[:, :, TS:TSP], 0.0)
    nc.vector.memset(dtuq[:, :, TS:TSP], 0.0)

    scan_q_idx = [0]

    wcur = load_weights(0)
    for l in range(L):
        wnext = load_weights(l + 1) if l + 1 < L else None
        wint_sb, wotr_sb, xpt_sb = wcur["wint"], wcur["wotr"], wcur["xpt"]
        dtpt_sb, cwd_sb, cb1_sb = wcur["dtpt"], wcur["cwd"], wcur["cb1"]
        dtpb_sb, asc_sb, dscd_sb = wcur["dtpb"], wcur["asc"], wcur["dscd"]

        # ---- LN ----
        hn = layernorm(wcur["lnrow"], wcur["lnbc"], BF16, "hn", outq=hnq)
        if l == 0:
            dbg("hn0", hnq[:, 0, :])

        # ---- in_proj xi tiles 0..7 (padded for conv) + z tiles ----
        for et in range(8):
            ps = pA.tile([128, TS], F32, name="ps")
            for kt in range(4):
                nc.tensor.matmul(ps, wint_sb[:, kt, et * 128:(et + 1) * 128],
                                 hnq[:, kt, :], start=(kt == 0), stop=(kt == 3))
            if et % 2 == 0:
                nc.scalar.copy(xi_pad[et][:, DC - 1:DC - 1 + TS], ps)
            else:
                nc.vector.tensor_copy(out=xi_pad[et][:, DC - 1:DC - 1 + TS],
                                      in_=ps)

        load_table(SET_SILU)

        # ---- causal depthwise conv1d + silu; x_proj accumulates per et ----
        # x_proj split: B/C rows [R:R+2N] in one PSUM group (finishes first,
        # feeds the broadcast DMA chain), dt rows [0:R] in another.
        psbc = pA.tile([2 * N, TS], F32, name="ps")
        psdt = pB.tile([R, TS], F32, name="pln")
        xits = []
        for et in range(8):
            psc = pA.tile([128, TS], F32, name="ps")
            for k in range(DC):
                nc.tensor.matmul(psc, cwd_sb[:, et, k, :],
                                 xi_pad[et][:, k:k + TS],
                                 start=(k == 0), stop=(k == DC - 1))
            if et < 4:
                xit = xiq[:, et, :]
            else:
                xio = work.tile([128, TS], BF16, name=f"xio{et % 2}")
                xit = xio
            tact(nc.scalar.activation(xit, psc, AF.Silu,
                                      bias=cb1_sb[:, et:et + 1]))
            nc.tensor.matmul(psbc, xpt_sb[:, et, R:R + 2 * N], xit,
                             start=(et == 0), stop=(et == 7))
            xits.append(xit)
        for et in range(8):
            nc.tensor.matmul(psdt, xpt_sb[:, et, 0:R], xits[et],
                             start=(et == 0), stop=(et == 7))
        if l == 0:
            dbg("xi0", xiq[:, 0, :])
        for zt in range(4):
            et = 8 + zt
            ps = pA.tile([128, TS], F32, name="ps")
            for kt in range(4):
                nc.tensor.matmul(ps, wint_sb[:, kt, et * 128:(et + 1) * 128],
                                 hnq[:, kt, :], start=(kt == 0), stop=(kt == 3))
            tact(nc.scalar.activation(zq[:, zt, :], ps, AF.Silu))
        if l == 0:
            dbg("zs0", zq[:, 0, :])

        xbc = work.tile([2 * N, TS], BF16, name="xbc")
        nc.vector.tensor_copy(out=xbc, in_=psbc)
        xdbl = work.tile([R, TS], BF16, name="xdbl")
        nc.vector.tensor_copy(out=xdbl, in_=psdt)
        if l == 0:
            dbg("xdbl", xdbl)

        # ---- stage B/C rows to DRAM, broadcast to all partitions ----
        bcst = dram.tile([2 * N, TS], BF16, name="bcst")
        nc.sync.dma_start(out=bcst, in_=xbc)
        for q in range(4):
            for half in range(2):
                r0 = half * N + NQ * q
                nc.sync.dma_start(
                    out=bcrep[:, r0:r0 + NQ, 0:TS],
                    in_=bcst[r0:r0 + NQ, :].unsqueeze(0).to_broadcast(
                        [128, NQ, TS]))
        if l == 0:
            dbg("brep", bcrep[:, 0, 0:TS])
            dbg("crep", bcrep[:, N, 0:TS])

        # ---- dt: softplus(dt_proj @ dt_raw + b) = ln(1 + exp(.)) ----
        load_table(SET_LNEXP)
        for kt in range(4):
            psd = pA.tile([128, TS], F32, name="ps")
            nc.tensor.matmul(psd, dtpt_sb[:, kt * 128:(kt + 1) * 128],
                             xdbl[:, :], start=True, stop=True)
            edt = work.tile([128, TS], BF16, name="edt")
            tact(nc.scalar.activation(edt, psd, AF.Exp, bias=dtpb_sb[:, kt:kt + 1]))
            tact(nc.scalar.activation(wq[:, kt, 0:TS], edt, AF.Ln,
                                      bias=onesf_sb[:, 0:1]))
            nc.vector.tensor_tensor(out=dtuq[:, kt, 0:TS], in0=wq[:, kt, 0:TS],
                                    in1=xiq[:, kt, :], op=OP.mult)
        if l == 0:
            dbg("w0", wq[:, 0, 0:TS])
            dbg("dtu0", dtuq[:, 0, 0:TS])

        # ---- scan (n-quad fused) + gating + chunked y-exchange ----
        ygq = work.tile([128, 4, TS], FP8, name="ygq")
        pso = None
        for kt in range(4):
            pyt = py.tile([128, TS], F32, name="py")
            # xi * D_skip folded into the PSUM accumulation via a diag matmul
            nc.tensor.matmul(pyt, dscd_sb[:, kt, :], xiq[:, kt, :],
                             start=True, stop=False)
            # produce all dBu quads for this kt up front (mostly on Pool,
            # running ahead of the DVE scan chain)
            dbus = []
            for qg in range(N // NQ):
                n0 = NQ * qg
                dBuq = scanb.tile([128, NQ, TSP], BF16, name="dBuq")
                on_dve = (qg == 0 and kt == 0) or (kt == 3 and qg >= 2)
                teng = nc.vector if on_dve else nc.gpsimd
                teng.tensor_tensor(
                    out=dBuq,
                    in0=dtuq[:, kt, :].unsqueeze(1).to_broadcast([128, NQ, TSP]),
                    in1=bcrep[:, n0:n0 + NQ, :], op=OP.mult)
                dbus.append(dBuq)
            for qg in range(N // NQ):
                n0 = NQ * qg
                dAq = scana.tile([128, NQ, TSP], BF16, name="dAq")
                nc.gpsimd.memset(dAq[:, :, TS:TSP], 0.0)
                for j in range(NQ):
                    col = kt * 16 + n0 + j
                    tact(nc.scalar.activation(dAq[:, j, 0:TS], wq[:, kt, 0:TS],
                                              AF.Exp,
                                              scale=asc_sb[:, col:col + 1]))
                hsq = scanh.tile([128, NQ, TSP], BF16, name="hsq")
                nc.vector.tensor_tensor_scan(
                    hsq[...].rearrange("p q t -> p (q t)"),
                    dAq[...].rearrange("p q t -> p (q t)"),
                    dbus[qg][...].rearrange("p q t -> p (q t)"),
                    0.0, OP.mult, OP.add)
                tmpq = scant.tile([128, NQ, TS], BF16, name="tmpq")
                nc.vector.tensor_tensor(
                    out=tmpq, in0=hsq[:, :, 0:TS],
                    in1=bcrep[:, N + n0:N + n0 + NQ, 0:TS], op=OP.mult)
                for j in range(NQ):
                    nc.tensor.matmul(pyt, identb_sb, tmpq[:, j, :],
                                     start=False,
                                     stop=(qg == N // NQ - 1 and j == NQ - 1))
                if l == 0 and kt == 0 and qg == 0:
                    dbg("dA00", dAq[:, 0, 0:TS])
                    dbg("dBu00", dbus[0][:, 0, 0:TS])
                    dbg("hs00", hsq[:, 0, 0:TS])
            # gating: yg = (y + xi * D) * silu(z)  (z pre-silu'd in zq)
            nc.vector.tensor_tensor(out=ygq[:, kt, :], in0=zq[:, kt, :],
                                    in1=pyt, op=OP.mult)
            if l == 0 and kt == 0:
                dbg("g10", ygq[:, 0, :])

            if kt % 2 == 1:
                c = kt // 2
                # exchange this chunk of gated y
                ccin = dram.tile([2 * 128, TS], FP8, name=f"ccin{c}")
                nc.sync.dma_start(
                    out=ccin.rearrange("(k p) t -> p k t", p=128),
                    in_=ygq[:, c * 2:c * 2 + 2, :])
                ccout = dram.tile([2 * 256, TS], FP8, name=f"ccout{c}")
                nc.gpsimd.collective_compute(
                    "AllGather", OP.bypass, replica_groups=GROUPS,
                    ins=[ccin[:, :]], outs=[ccout[:, :]],
                )
                ygf8 = work.tile([128, 4, TS], FP8, name=f"yg8{c}")
                ccv = ccout.rearrange("(kd p) t -> p kd t", p=128)
                for kd in range(4):
                    nc.sync.dma_start(out=ygf8[:, kd, :], in_=ccv[:, kd, :])
                if pso is None:
                    pso = [pA.tile([128, TS], F32, name="ps"),
                           pA.tile([128, TS], F32, name="ps"),
                           pB.tile([128, TS], F32, name="pln"),
                           pB.tile([128, TS], F32, name="pln")]
                for b in range(2):
                    for mt in range(4):
                        nc.tensor.matmul(
                            pso[mt],
                            wotr_sb[:, c * 4 + 2 * b:c * 4 + 2 * b + 2,
                                    mt * 128:(mt + 1) * 128],
                            ygf8[:, 2 * b:2 * b + 2, :],
                            start=(c == 0 and b == 0),
                            stop=(c == CC_CHUNKS - 1 and b == 1),
                            perf_mode=mybir.MatmulPerfMode.DoubleRow)

        # ---- residual ----
        for mt in range(4):
            nc.vector.tensor_tensor(out=h[mt], in0=h[mt], in1=pso[mt], op=OP.add)
        if l == 0:
            dbg("hl0", h[0])

        wcur = wnext

    # ---- final LN ----
    hnf = layernorm(nwrow_sb, nbc_sb, F32, "hnf")

    # ---- transpose + store ([TS, DM]; upsample happens on the host) ----
    for ct in range(4):
        hT = work.tile([128, DM], F32, name=f"hT{ct}")
        for kt in range(4):
            pt = pA.tile([128, 128], F32, name="ps")
            nc.tensor.transpose(pt, hnf[kt][:, ct * 128:(ct + 1) * 128], ident_sb)
            nc.vector.tensor_copy(out=hT[:, kt * 128:(kt + 1) * 128], in_=pt)
        nc.sync.dma_start(out=out[ct * 128:(ct + 1) * 128, :], in_=hT)


def _build_nc():
    nc = bacc.Bacc("TRN2", num_devices=NC_CORES)
    ins = {}

    def din(name, shape, dt):
        ins[name] = nc.dram_tensor(name, list(shape), dt, kind="ExternalInput")

    din("xt", (128, T + K - 1), BF16)
    din("w1t", (K, 128, DM), BF16)
    din("cb", (128, 4), F32)
    din("ident", (128, 128), F32)
    din("identb", (128, 128), BF16)
    din("nwrow", (1, DM), BF16)
    din("nbc", (128, 4), F32)
    din("wint", (L, DM, DI + DH), BF16)
    din("wotr", (L, DI, DM), FP8)
    din("xpt", (L, DI, R + 2 * N), BF16)
    din("dtpt", (L, R, DH), BF16)
    din("cwdiag", (L, 8, DC, 128, 128), BF16)
    din("cb1d", (L, 128, 8), F32)
    din("dtpb", (L, 128, 4), F32)
    din("asc", (L, 128, 64), F32)
    din("dscdiag", (L, 4, 128, 128), BF16)
    din("lnrow", (L, 1, DM), BF16)
    din("lnbc", (L, 128, 4), F32)
    out = nc.dram_tensor("out", [TS, DM], F32, kind="ExternalOutput")

    dbgs = [] if _DEBUG else None
    with ExitStack() as ctx:
        tc = ctx.enter_context(tile.TileContext(nc))
        _emit(ctx, tc, ins, out, dbgs)
    nc.compile()
    _CACHE["dbgs"] = dbgs
    return nc


def _prep_core_inputs(c, inputs):
    b, m = c // 2, c % 2
    bf = lambda a: np.ascontiguousarray(a).astype(NPBF16)
    f32 = lambda a: np.ascontiguousarray(a).astype(np.float32)

    x = np.asarray(inputs["x"], np.float32)
    xt = np.zeros((128, T + K - 1), np.float32)
    xt[:, K - 1:] = x[b].T
    w1t = np.asarray(inputs["conv_w"], np.float32).transpose(2, 1, 0)  # [K,F,DM]
    cb = np.asarray(inputs["conv_b"], np.float32).reshape(4, 128).T
    ident = np.eye(128, dtype=np.float32)
    nwrow = np.asarray(inputs["norm_w"], np.float32).reshape(1, DM)
    nbc = np.asarray(inputs["norm_b"], np.float32).reshape(4, 128).T

    # per-core DI channel permutation: own half first
    own = np.arange(m * DH, (m + 1) * DH)
    oth = np.arange((1 - m) * DH, (2 - m) * DH)
    perm = np.concatenate([own, oth])

    in_w = np.asarray(inputs["in_proj_w"], np.float32)    # [L, 2*DI, DM]
    wint = np.empty((L, DM, DI + DH), np.float32)
    for l in range(L):
        wtp = in_w[l].T                                   # [DM, 2*DI]
        wint[l, :, :DI] = wtp[:, perm]                    # xi, permuted
        wint[l, :, DI:] = wtp[:, DI + own]                # z own half
    # out_proj rows in chunk-arrival order (unpermuted channels):
    # chunk c: [h0 ch 256c..256c+256, h1 ch 512+256c..512+256c+256]
    wot = np.asarray(inputs["out_proj_w"], np.float32).transpose(0, 2, 1)  # [L,DI,DM]
    KT_PER_CC = 4 // CC_CHUNKS
    row_order = []
    for cc in range(CC_CHUNKS):
        w0 = cc * KT_PER_CC * 128
        row_order.extend(range(w0, w0 + KT_PER_CC * 128))
        row_order.extend(range(DH + w0, DH + w0 + KT_PER_CC * 128))
    wotr = wot[:, row_order, :]
    xpt = np.asarray(inputs["x_proj_w"], np.float32).transpose(0, 2, 1)[:, perm, :]
    dtpt = np.asarray(inputs["dt_proj_w"], np.float32).transpose(0, 2, 1)[:, :, own]
    cw1d = np.asarray(inputs["conv1d_w"], np.float32)[:, perm, :]
    cwdiag = np.zeros((L, 8, DC, 128, 128), np.float32)
    ii = np.arange(128)
    for l in range(L):
        for et in range(8):
            for k in range(DC):
                cwdiag[l, et, k, ii, ii] = cw1d[l, et * 128:(et + 1) * 128, k]
    cb1d = np.asarray(inputs["conv1d_b"], np.float32)[:, perm].reshape(L, 8, 128)
    cb1d = cb1d.transpose(0, 2, 1)
    dtpb = np.asarray(inputs["dt_proj_b"], np.float32)[:, own].reshape(L, 4, 128)
    dtpb = dtpb.transpose(0, 2, 1)
    A = -np.exp(np.asarray(inputs["A_log"], np.float32))[:, own, :]  # [L, DH, N]
    # asc[l, p, 16*kt + n] = A[l, kt*128 + p, n]
    asc = A.reshape(L, 4, 128, N).transpose(0, 2, 1, 3).reshape(L, 128, 64)
    dval = np.asarray(inputs["D_skip"], np.float32)[:, own]
    dscdiag = np.zeros((L, 4, 128, 128), np.float32)
    for l in range(L):
        for kt in range(4):
            dscdiag[l, kt, ii, ii] = dval[l, kt * 128:(kt + 1) * 128]
    lnrow = np.asarray(inputs["ln_w"], np.float32).reshape(L, 1, DM)
    lnbc = np.asarray(inputs["ln_b"], np.float32).reshape(L, 4, 128)
    lnbc = lnbc.transpose(0, 2, 1)

    f8 = lambda a: np.ascontiguousarray(a).astype(ml_dtypes.float8_e4m3)
    return dict(
        xt=bf(xt), w1t=bf(w1t), cb=f32(cb), ident=ident,
        identb=bf(np.eye(128, dtype=np.float32)), nwrow=bf(nwrow),
        nbc=f32(nbc),
        wint=bf(wint), wotr=f8(wotr), xpt=bf(xpt), dtpt=bf(dtpt),
        cwdiag=bf(cwdiag), cb1d=f32(cb1d), dtpb=f32(dtpb), asc=f32(asc),
        dscdiag=bf(dscdiag), lnrow=bf(lnrow), lnbc=f32(lnbc),
    )


def kernel(trace=False, **inputs):
    if "nc" not in _CACHE:
        _CACHE["nc"] = _build_nc()
    nc = _CACHE["nc"]
    in_maps = [_prep_core_inputs(c, inputs) for c in range(NC_CORES)]
    res = run_bass_kernel_spmd(nc, in_maps, list(range(NC_CORES)), trace=trace)
    out = np.stack([
        np.repeat(np.asarray(res.results[2 * b]["out"], np.float32),
                  STRIDE, axis=0)[:T]
        for b in range(B)])
    _CACHE["last_result"] = res
    return out


# revision 52
# speedup vs baseline: 146.4680x; 1.0005x over previous
"""Trainium2 Bass kernel for the ConvBranch (Mamba-style) model.

Sharding: 8 cores = 4 batches x 2 DI-halves.
  core c -> batch b = c//2, half m = c%2 (owns DI channels [m*512,(m+1)*512)).
Dense matmuls (in_proj/conv/x_proj/out_proj) are replicated within a pair;
the selective-scan trio is sharded by DI-half; gated y halves are exchanged
with chunked AllGathers per layer (overlapped with the scan).

Scan layout (n-quad fused): per (kt, q) process 4 state indices in one
[128, 4, TS+1] tile; a zero separator column between n-segments resets the
scan state, so one tensor_tensor_scan covers 4 independent recurrences.
dBu / tmp are single quad TT ops with the B/C rows broadcast via stride-0
access patterns. Scans and tmp run on DVE (HW GPSIMD cannot execute
TensorScalarPtr or touch PSUM); most dBu quads run on GpSimd, produced
ahead of the scan chain. dA exps on Act; xi*D folds into the y-PSUM via a
diagonal matmul.

The gated-y exchange is fp8(e4m3): two AllGathers per layer ([kt0,kt1],
[kt2,kt3]), the first hidden behind the second half of the scan. out_proj
consumes the fp8 payload directly with DoubleRow fp8 matmuls (fp8 wotr).

Activation tables are loaded manually; two set switches per layer
(SILU block, then LNEXP block).

Output is [TS, DM] per core; the x4 repeat-interleave upsample happens on
the host.
"""

import sys

sys.path.insert(0, "/opt/trn_rl_repo")

from contextlib import ExitStack

import numpy as np
import ml_dtypes

import concourse.bass as bass
import concourse.bacc as bacc
import concourse.tile as tile
from concourse import mybir
from concourse.bass_utils import run_bass_kernel_spmd
from concourse.tile_rust import add_dep_helper

F32 = mybir.dt.float32
BF16 = mybir.dt.bfloat16
FP8 = mybir.dt.float8e4
NPBF16 = ml_dtypes.bfloat16
AF = mybir.ActivationFunctionType
OP = mybir.AluOpType

B, T, F = 4, 2048, 128
DM, L, STRIDE, KF = 512, 4, 4, 2
N, DC, E = 16, 4, 2
DI = E * DM            # 1024
R = (DM + 15) // 16    # 32
K = KF * STRIDE        # 8
TS = T // STRIDE       # 512
TSP = TS + 1           # +1 separator col for n-fused scans
EPS = 1e-5
DH = DI // 2           # 512 channels per core half
NC_CORES = 8
GROUPS = [[0, 1], [2, 3], [4, 5], [6, 7]]
NQ = 4                 # n-quad group size

# act_info.json set ids (gen3): 6 = ln+exp(+square/copy), 10 = gelu, 18 = silu
SET_LNEXP, SET_GELU, SET_SILU = 6, 10, 18

# dBu quad-TT placement (Pool cannot run scans or stt on HW; TT runs at
# 0.42 efficiency there). dBu only needs dtu+bcrep so Pool can produce it
# ahead of the scan chain; tmp stays on DVE (it feeds PE directly).
CC_CHUNKS = 2          # y-exchange chunks per layer (kt tiles per chunk = 2)

_CACHE = {}
_DEBUG = False


def _emit(ctx, tc, ins, out, dbgs=None):
    nc = tc.nc

    def dbg(name, ap):
        if dbgs is None:
            return
        t = nc.dram_tensor("dbg_" + name, list(ap.shape), ap.dtype,
                           kind="ExternalOutput")
        nc.sync.dma_start(out=t[...], in_=ap)
        dbgs.append("dbg_" + name)

    # Manual activation-table management (same-engine ordering-only edges).
    _tbl = {"load": None, "since": []}

    def load_table(set_id):
        inst = nc.scalar.add_instruction(
            mybir.InstLoadActFuncSet(
                name=nc.get_next_instruction_name(),
                act_func_set_id=set_id, ins=[], outs=[]))
        for p in _tbl["since"]:
            add_dep_helper(inst.ins, p, sync=False, reason="act-table order")
        if _tbl["load"] is not None:
            add_dep_helper(inst.ins, _tbl["load"], sync=False,
                           reason="act-table order")
        _tbl["since"] = []
        _tbl["load"] = inst.ins

    def tact(res):
        if _tbl["load"] is not None:
            add_dep_helper(res.ins, _tbl["load"], sync=False,
                           reason="act-table order")
        _tbl["since"].append(res.ins)
        return res

    consts = ctx.enter_context(tc.tile_pool(name="consts", bufs=1))
    wpool1 = ctx.enter_context(tc.tile_pool(name="wpool1", bufs=1))
    work = ctx.enter_context(tc.tile_pool(name="work", bufs=1))
    scana = ctx.enter_context(tc.tile_pool(name="scana", bufs=3))
    scanh = ctx.enter_context(tc.tile_pool(name="scanh", bufs=3))
    scanb = ctx.enter_context(tc.tile_pool(name="scanb", bufs=4))
    scant = ctx.enter_context(tc.tile_pool(name="scant", bufs=2))
    pA = ctx.enter_context(tc.tile_pool(name="pA", bufs=4, space="PSUM"))
    pB = ctx.enter_context(tc.tile_pool(name="pB", bufs=2, space="PSUM"))
    py = ctx.enter_context(tc.tile_pool(name="py", bufs=2, space="PSUM"))
    dram = ctx.enter_context(tc.tile_pool(name="dram", bufs=1, space="DRAM"))

    # ---- persistent constants ----
    xt_sb = wpool1.tile([128, T + K - 1], BF16, name="wotr")
    nc.sync.dma_start(out=xt_sb, in_=ins["xt"][:, :])
    w1t_sb = wpool1.tile([128, K, DM], BF16, name="cwd")
    nc.sync.dma_start(out=w1t_sb, in_=ins["w1t"].rearrange("k f m -> f k m"))
    cb_sb = consts.tile([128, 4], F32)
    nc.sync.dma_start(out=cb_sb, in_=ins["cb"][:, :])
    ident_sb = consts.tile([128, 128], F32)
    nc.sync.dma_start(out=ident_sb, in_=ins["ident"][:, :])
    identb_sb = consts.tile([128, 128], BF16)
    nc.sync.dma_start(out=identb_sb, in_=ins["identb"][:, :])
    nwrow_sb = consts.tile([1, DM], BF16)
    nc.sync.dma_start(out=nwrow_sb, in_=ins["nwrow"][:, :])
    nbc_sb = consts.tile([128, 4], F32)
    nc.sync.dma_start(out=nbc_sb, in_=ins["nbc"][:, :])
    ones128_sb = consts.tile([128, 1], BF16)
    nc.vector.memset(ones128_sb, 1.0)
    eps1_sb = consts.tile([1, 1], F32)
    nc.vector.memset(eps1_sb, EPS)
    onesf_sb = consts.tile([128, 1], F32)
    nc.vector.memset(onesf_sb, 1.0)

    # B/C broadcast target: [128, 2N, TSP]; separator cols zeroed once.
    bcrep = consts.tile([128, 2 * N, TSP], BF16, name="bcrep")
    nc.vector.memset(bcrep[:, :, TS:TSP], 0.0)

    # residual stream h: 4 persistent fp32 tiles [128, TS]
    h = [consts.tile([128, TS], F32, name=f"h{kt}") for kt in range(4)]

    # ---- front conv + gelu ----
    load_table(SET_GELU)
    for mt in range(4):
        ps = pA.tile([128, TS], F32, name="ps")
        for k in range(K):
            nc.tensor.matmul(
                ps,
                w1t_sb[:, k, mt * 128:(mt + 1) * 128],
                xt_sb[:, k:k + T:STRIDE],
                start=(k == 0),
                stop=(k == K - 1),
            )
        tact(nc.scalar.activation(h[mt], ps, AF.Gelu, bias=cb_sb[:, mt:mt + 1]))
    load_table(SET_LNEXP)

    def load_weights(l):
        w = {}

        def wt(name, shape, dt, src, pool=wpool1):
            t = pool.tile(shape, dt, name=name)
            nc.sync.dma_start(out=t, in_=src)
            w[name] = t

        wt("wint", [128, 4, DI + DH], BF16,
           ins["wint"][l].rearrange("(kt p) e -> p kt e", p=128))
        wt("wotr", [128, 8, DM], FP8,
           ins["wotr"][l].rearrange("(kd p) o -> p kd o", p=128))
        wt("xpt", [128, 8, R + 2 * N], BF16,
           ins["xpt"][l].rearrange("(kd p) e -> p kd e", p=128))
        wt("dtpt", [32, DH], BF16, ins["dtpt"][l])
        wt("cwd", [128, 8, DC, 128], BF16,
           ins["cwdiag"][l].rearrange("e k p q -> p e k q"))
        wt("cb1", [128, 8], F32, ins["cb1d"][l])
        wt("dtpb", [128, 4], F32, ins["dtpb"][l])
        wt("asc", [128, 64], F32, ins["asc"][l])
        wt("dscd", [128, 4, 128], BF16,
           ins["dscdiag"][l].rearrange("k p q -> p k q"))
        wt("lnrow", [1, DM], BF16, ins["lnrow"][l])
        wt("lnbc", [128, 4], F32, ins["lnbc"][l])
        return w

    def layernorm(lnrow, lnbc, out_dtype, name, outq=None):
        """LN over the feature (partition) dim of h; affine folded into the
        rank-1 PE broadcasts: hn = (h*A + lb) - Bc, A = lw*rstd, Bc = lw*c.
        If outq is given, write kt slices into outq[:, kt, :]."""
        st_m = pA.tile([128, TS], F32, name="ps")
        st_q = pA.tile([128, TS], F32, name="ps")
        for kt in range(4):
            hbt = work.tile([128, TS], BF16, name=f"hb{kt % 2}")
            nc.gpsimd.tensor_copy(out=hbt, in_=h[kt])
            sq = work.tile([128, TS], BF16, name="sq")
            nc.scalar.activation(sq, h[kt], AF.Square)
            nc.tensor.matmul(st_m[0:1, :], ones128_sb, hbt,
                             start=(kt == 0), stop=(kt == 3))
            nc.tensor.matmul(st_q[0:1, :], ones128_sb, sq,
                             start=(kt == 0), stop=(kt == 3))
        ms = work.tile([1, 2 * TS], F32, name="ms")
        nc.vector.tensor_scalar(ms[:, 0:TS], st_m[0:1, :], 1.0 / DM, None, OP.mult)
        mu2 = work.tile([1, TS], F32, name="mu2")
        nc.vector.tensor_tensor(out=mu2, in0=ms[:, 0:TS], in1=ms[:, 0:TS],
                                op=OP.mult)
        var = work.tile([1, TS], F32, name="var")
        nc.vector.scalar_tensor_tensor(var, st_q[0:1, :], 1.0 / DM, mu2,
                                       OP.mult, OP.subtract)
        # rstd = exp(-0.5 * ln(var + eps)); c = mu * rstd   (packed [1, 2*TS])
        rc = work.tile([1, 2 * TS], F32, name="rc")
        lnv = work.tile([1, TS], F32, name="lnv")
        tact(nc.scalar.activation(lnv, var, AF.Ln, bias=eps1_sb[:, 0:1]))
        tact(nc.scalar.activation(rc[:, 0:TS], lnv, AF.Exp, scale=-0.5))
        nc.vector.tensor_tensor(out=rc[:, TS:2 * TS], in0=ms[:, 0:TS],
                                in1=rc[:, 0:TS], op=OP.mult)
        rcb = work.tile([1, 2 * TS], BF16, name="rcb")
        nc.scalar.copy(rcb, rc)
        outs = []
        for kt in range(4):
            sl = slice(kt * 128, (kt + 1) * 128)
            pa = pB.tile([128, TS], F32, name="pln")
            nc.tensor.matmul(pa, lnrow[0:1, sl], rcb[:, 0:TS],
                             start=True, stop=True)
            pbc = pB.tile([128, TS], F32, name="pln")
            nc.tensor.matmul(pbc, lnrow[0:1, sl], rcb[:, TS:2 * TS],
                             start=True, stop=True)
            t1 = work.tile([128, TS], F32, name="lnt1")
            nc.vector.tensor_tensor(out=t1, in0=h[kt], in1=pa, op=OP.mult)
            o = outq[:, kt, :] if outq is not None else None
            if o is None:
                ot = work.tile([128, TS], out_dtype, name=f"{name}{kt}")
                o = ot
            nc.vector.scalar_tensor_tensor(o, t1, lnbc[:, kt:kt + 1], pbc,
                                           OP.add, OP.subtract)
            outs.append(o)
        return outs

    # zero the causal pads of the conv input tiles once; the per-layer copy
    # only writes cols [DC-1:).
    xi_pad = [work.tile([128, DC - 1 + TS], BF16, name=f"xipad{et}")
              for et in range(8)]
    for et in range(8):
        nc.vector.memset(xi_pad[et][:, 0:DC - 1], 0.0)

    # persistent quad tiles
    hnq = consts.tile([128, 4, TS], BF16, name="hnq")
    xiq = consts.tile([128, 4, TS], BF16, name="xiq")   # silu(conv), own half
    zq = consts.tile([128, 4, TS], BF16, name="zq")     # silu(z)
    wq = consts.tile([128, 4, TSP], BF16, name="wq")    # softplus dt
    dtuq = consts.tile([128, 4, TSP], BF16, name="dtuq")
    nc.vector.memset(wq[:, :, TS:TSP], 0.0)
    nc.vector.memset(dtuq[:, :, TS:TSP], 0.0)

    scan_q_idx = [0]

    wcur = load_weights(0)
    for l in range(L):
        wnext = load_weights(l + 1) if l + 1 < L else None
        wint_sb, wotr_sb, xpt_sb = wcur["wint"], wcur["wotr"], wcur["xpt"]
        dtpt_sb, cwd_sb, cb1_sb = wcur["dtpt"], wcur["cwd"], wcur["cb1"]
        dtpb_sb, asc_sb, dscd_sb = wcur["dtpb"], wcur["asc"], wcur["dscd"]

        # ---- LN ----
        hn = layernorm(wcur["lnrow"], wcur["lnbc"], BF16, "hn", outq=hnq)
        if l == 0:
            dbg("hn0", hnq[:, 0, :])

        # ---- in_proj xi tiles 0..7 (padded for conv) + z tiles ----
        for et in range(8):
            ps = pA.tile([128, TS], F32, name="ps")
            for kt in range(4):
                nc.tensor.matmul(ps, wint_sb[:, kt, et * 128:(et + 1) * 128],
                                 hnq[:, kt, :], start=(kt == 0), stop=(kt == 3))
            if et % 2 == 0:
                nc.scalar.copy(xi_pad[et][:, DC - 1:DC - 1 + TS], ps)
            else:
                nc.vector.tensor_copy(out=xi_pad[et][:, DC - 1:DC - 1 + TS],
                                      in_=ps)

        load_table(SET_SILU)

        # ---- causal depthwise conv1d + silu; x_proj accumulates per et ----
        # x_proj split: B/C rows [R:R+2N] in one PSUM group (finishes first,
        # feeds the broadcast DMA chain), dt rows [0:R] in another.
        psbc = pA.tile([2 * N, TS], F32, name="ps")
        psdt = pB.tile([R, TS], F32, name="pln")
        xits = []
        for et in range(8):
            psc = pA.tile([128, TS], F32, name="ps")
            for k in range(DC):
                nc.tensor.matmul(psc, cwd_sb[:, et, k, :],
                                 xi_pad[et][:, k:k + TS],
                                 start=(k == 0), stop=(k == DC - 1))
            if et < 4:
                xit = xiq[:, et, :]
            else:
                xio = work.tile([128, TS], BF16, name=f"xio{et % 2}")
                xit = xio
            tact(nc.scalar.activation(xit, psc, AF.Silu,
                                      bias=cb1_sb[:, et:et + 1]))
            nc.tensor.matmul(psbc, xpt_sb[:, et, R:R + 2 * N], xit,
                             start=(et == 0), stop=(et == 7))
            xits.append(xit)
        for et in range(8):
            nc.tensor.matmul(psdt, xpt_sb[:, et, 0:R], xits[et],
                             start=(et == 0), stop=(et == 7))
        if l == 0:
            dbg("xi0", xiq[:, 0, :])
        for zt in range(4):
            et = 8 + zt
            ps = pA.tile([128, TS], F32, name="ps")
            for kt in range(4):
                nc.tensor.matmul(ps, wint_sb[:, kt, et * 128:(et + 1) * 128],
                                 hnq[:, kt, :], start=(kt == 0), stop=(kt == 3))
            tact(nc.scalar.activation(zq[:, zt, :], ps, AF.Silu))
        if l == 0:
            dbg("zs0", zq[:, 0, :])

        xbc = work.tile([2 * N, TS], BF16, name="xbc")
        nc.vector.tensor_copy(out=xbc, in_=psbc)
        xdbl = work.tile([R, TS], BF16, name="xdbl")
        nc.vector.tensor_copy(out=xdbl, in_=psdt)
        if l == 0:
            dbg("xdbl", xdbl)

        # ---- stage B/C rows to DRAM, broadcast to all partitions ----
        bcst = dram.tile([2 * N, TS], BF16, name="bcst")
        nc.sync.dma_start(out=bcst, in_=xbc)
        for q in range(4):
            for half in range(2):
                r0 = half * N + NQ * q
                nc.sync.dma_start(
                    out=bcrep[:, r0:r0 + NQ, 0:TS],
                    in_=bcst[r0:r0 + NQ, :].unsqueeze(0).to_broadcast(
                        [128, NQ, TS]))
        if l == 0:
            dbg("brep", bcrep[:, 0, 0:TS])
            dbg("crep", bcrep[:, N, 0:TS])

        # ---- dt: softplus(dt_proj @ dt_raw + b) = ln(1 + exp(.)) ----
        load_table(SET_LNEXP)
        for kt in range(4):
            psd = pA.tile([128, TS], F32, name="ps")
            nc.tensor.matmul(psd, dtpt_sb[:, kt * 128:(kt + 1) * 128],
                             xdbl[:, :], start=True, stop=True)
            edt = work.tile([128, TS], BF16, name="edt")
            tact(nc.scalar.activation(edt, psd, AF.Exp, bias=dtpb_sb[:, kt:kt + 1]))
            tact(nc.scalar.activation(wq[:, kt, 0:TS], edt, AF.Ln,
                                      bias=onesf_sb[:, 0:1]))
            nc.vector.tensor_tensor(out=dtuq[:, kt, 0:TS], in0=wq[:, kt, 0:TS],
                                    in1=xiq[:, kt, :], op=OP.mult)
        if l == 0:
            dbg("w0", wq[:, 0, 0:TS])
            dbg("dtu0", dtuq[:, 0, 0:TS])

        # ---- scan (n-quad fused) + gating + chunked y-exchange ----
        ygq = work.tile([128, 4, TS], FP8, name="ygq")
        pso = None
        for kt in range(4):
            pyt = py.tile([128, TS], F32, name="py")
            # xi * D_skip folded into the PSUM accumulation via a diag matmul
            nc.tensor.matmul(pyt, dscd_sb[:, kt, :], xiq[:, kt, :],
                             start=True, stop=False)
            # produce all dBu quads for this kt up front (mostly on Pool,
            # running ahead of the DVE scan chain)
            dbus = []
            for qg in range(N // NQ):
                n0 = NQ * qg
                dBuq = scanb.tile([128, NQ, TSP], BF16, name="dBuq")
                on_dve = (qg == 0 and kt == 0) or (kt == 3 and qg >= 2)
                teng = nc.vector if on_dve else nc.gpsimd
                teng.tensor_tensor(
                    out=dBuq,
                    in0=dtuq[:, kt, :].unsqueeze(1).to_broadcast([128, NQ, TSP]),
                    in1=bcrep[:, n0:n0 + NQ, :], op=OP.mult)
                dbus.append(dBuq)
            for qg in range(N // NQ):
                n0 = NQ * qg
                dAq = scana.tile([128, NQ, TSP], BF16, name="dAq")
                nc.gpsimd.memset(dAq[:, :, TS:TSP], 0.0)
                for j in range(NQ):
                    col = kt * 16 + n0 + j
                    tact(nc.scalar.activation(dAq[:, j, 0:TS], wq[:, kt, 0:TS],
                                              AF.Exp,
                                              scale=asc_sb[:, col:col + 1]))
                hsq = scanh.tile([128, NQ, TSP], BF16, name="hsq")
                nc.vector.tensor_tensor_scan(
                    hsq[...].rearrange("p q t -> p (q t)"),
                    dAq[...].rearrange("p q t -> p (q t)"),
                    dbus[qg][...].rearrange("p q t -> p (q t)"),
                    0.0, OP.mult, OP.add)
                tmpq = scant.tile([128, NQ, TS], BF16, name="tmpq")
                nc.vector.tensor_tensor(
                    out=tmpq, in0=hsq[:, :, 0:TS],
                    in1=bcrep[:, N + n0:N + n0 + NQ, 0:TS], op=OP.mult)
                for j in range(NQ):
                    nc.tensor.matmul(pyt, identb_sb, tmpq[:, j, :],
                                     start=False,
                                     stop=(qg == N // NQ - 1 and j == NQ - 1))
                if l == 0 and kt == 0 and qg == 0:
                    dbg("dA00", dAq[:, 0, 0:TS])
                    dbg("dBu00", dbus[0][:, 0, 0:TS])
                    dbg("hs00", hsq[:, 0, 0:TS])
            # gating: yg = (y + xi * D) * silu(z)  (z pre-silu'd in zq)
            nc.vector.tensor_tensor(out=ygq[:, kt, :], in0=zq[:, kt, :],
                                    in1=pyt, op=OP.mult)
            if l == 0 and kt == 0:
                dbg("g10", ygq[:, 0, :])

            if kt % 2 == 1:
                c = kt // 2
                # exchange this chunk of gated y
                ccin = dram.tile([2 * 128, TS], FP8, name=f"ccin{c}")
                ccr = ccin.rearrange("(k p) t -> p k t", p=128)
                for j in range(2):
                    nc.sync.dma_start(out=ccr[:, j, :],
                                      in_=ygq[:, c * 2 + j, :])
                ccout = dram.tile([2 * 256, TS], FP8, name=f"ccout{c}")
                nc.gpsimd.collective_compute(
                    "AllGather", OP.bypass, replica_groups=GROUPS,
                    ins=[ccin[:, :]], outs=[ccout[:, :]],
                )
                ygf8 = work.tile([128, 4, TS], FP8, name=f"yg8{c}")
                ccv = ccout.rearrange("(kd p) t -> p kd t", p=128)
                for kd in range(4):
                    nc.sync.dma_start(out=ygf8[:, kd, :], in_=ccv[:, kd, :])
                if pso is None:
                    pso = [pA.tile([128, TS], F32, name="ps"),
                           pA.tile([128, TS], F32, name="ps"),
                           pB.tile([128, TS], F32, name="pln"),
                           pB.tile([128, TS], F32, name="pln")]
                for b in range(2):
                    for mt in range(4):
                        nc.tensor.matmul(
                            pso[mt],
                            wotr_sb[:, c * 4 + 2 * b:c * 4 + 2 * b + 2,
                                    mt * 128:(mt + 1) * 128],
                            ygf8[:, 2 * b:2 * b + 2, :],
                            start=(c == 0 and b == 0),
                            stop=(c == CC_CHUNKS - 1 and b == 1),
                            perf_mode=mybir.MatmulPerfMode.DoubleRow)

        # ---- residual ----
        for mt in range(4):
            nc.vector.tensor_tensor(out=h[mt], in0=h[mt], in1=pso[mt], op=OP.add)
        if l == 0:
            dbg("hl0", h[0])

        wcur = wnext

    # ---- final LN ----
    hnf = layernorm(nwrow_sb, nbc_sb, F32, "hnf")

    # ---- transpose + store ([TS, DM]; upsample happens on the host) ----
    for ct in range(4):
        hT = work.tile([128, DM], F32, name=f"hT{ct}")
        for kt in range(4):
            pt = pA.tile([128, 128], F32, name="ps")
            nc.tensor.transpose(pt, hnf[kt][:, ct * 128:(ct + 1) * 128], ident_sb)
            nc.vector.tensor_copy(out=hT[:, kt * 128:(kt + 1) * 128], in_=pt)
        nc.sync.dma_start(out=out[ct * 128:(ct + 1) * 128, :], in_=hT)


def _build_nc():
    nc = bacc.Bacc("TRN2", num_devices=NC_CORES)
    ins = {}

    def din(name, shape, dt):
        ins[name] = nc.dram_tensor(name, list(shape), dt, kind="ExternalInput")

    din("xt", (128, T + K - 1), BF16)
    din("w1t", (K, 128, DM), BF16)
    din("cb", (128, 4), F32)
    din("ident", (128, 128), F32)
    din("identb", (128, 128), BF16)
    din("nwrow", (1, DM), BF16)
    din("nbc", (128, 4), F32)
    din("wint", (L, DM, DI + DH), BF16)
    din("wotr", (L, DI, DM), FP8)
    din("xpt", (L, DI, R + 2 * N), BF16)
    din("dtpt", (L, R, DH), BF16)
    din("cwdiag", (L, 8, DC, 128, 128), BF16)
    din("cb1d", (L, 128, 8), F32)
    din("dtpb", (L, 128, 4), F32)
    din("asc", (L, 128, 64), F32)
    din("dscdiag", (L, 4, 128, 128), BF16)
    din("lnrow", (L, 1, DM), BF16)
    din("lnbc", (L, 128, 4), F32)
    out = nc.dram_tensor("out", [TS, DM], F32, kind="ExternalOutput")

    dbgs = [] if _DEBUG else None
    with ExitStack() as ctx:
        tc = ctx.enter_context(tile.TileContext(nc))
        _emit(ctx, tc, ins, out, dbgs)
    nc.compile()
    _CACHE["dbgs"] = dbgs
    return nc


def _prep_core_inputs(c, inputs):
    b, m = c // 2, c % 2
    bf = lambda a: np.ascontiguousarray(a).astype(NPBF16)
    f32 = lambda a: np.ascontiguousarray(a).astype(np.float32)

    x = np.asarray(inputs["x"], np.float32)
    xt = np.zeros((128, T + K - 1), np.float32)
    xt[:, K - 1:] = x[b].T
    w1t = np.asarray(inputs["conv_w"], np.float32).transpose(2, 1, 0)  # [K,F,DM]
    cb = np.asarray(inputs["conv_b"], np.float32).reshape(4, 128).T
    ident = np.eye(128, dtype=np.float32)
    nwrow = np.asarray(inputs["norm_w"], np.float32).reshape(1, DM)
    nbc = np.asarray(inputs["norm_b"], np.float32).reshape(4, 128).T

    # per-core DI channel permutation: own half first
    own = np.arange(m * DH, (m + 1) * DH)
    oth = np.arange((1 - m) * DH, (2 - m) * DH)
    perm = np.concatenate([own, oth])

    in_w = np.asarray(inputs["in_proj_w"], np.float32)    # [L, 2*DI, DM]
    wint = np.empty((L, DM, DI + DH), np.float32)
    for l in range(L):
        wtp = in_w[l].T                                   # [DM, 2*DI]
        wint[l, :, :DI] = wtp[:, perm]                    # xi, permuted
        wint[l, :, DI:] = wtp[:, DI + own]                # z own half
    # out_proj rows in chunk-arrival order (unpermuted channels):
    # chunk c: [h0 ch 256c..256c+256, h1 ch 512+256c..512+256c+256]
    wot = np.asarray(inputs["out_proj_w"], np.float32).transpose(0, 2, 1)  # [L,DI,DM]
    KT_PER_CC = 4 // CC_CHUNKS
    row_order = []
    for cc in range(CC_CHUNKS):
        w0 = cc * KT_PER_CC * 128
        row_order.extend(range(w0, w0 + KT_PER_CC * 128))
        row_order.extend(range(DH + w0, DH + w0 + KT_PER_CC * 128))
    wotr = wot[:, row_order, :]
    xpt = np.asarray(inputs["x_proj_w"], np.float32).transpose(0, 2, 1)[:, perm, :]
    dtpt = np.asarray(inputs["dt_proj_w"], np.float32).transpose(0, 2, 1)[:, :, own]
    cw1d = np.asarray(inputs["conv1d_w"], np.float32)[:, perm, :]
    cwdiag = np.zeros((L, 8, DC, 128, 128), np.float32)
    ii = np.arange(128)
    for l in range(L):
        for et in range(8):
            for k in range(DC):
                cwdiag[l, et, k, ii, ii] = cw1d[l, et * 128:(et + 1) * 128, k]
    cb1d = np.asarray(inputs["conv1d_b"], np.float32)[:, perm].reshape(L, 8, 128)
    cb1d = cb1d.transpose(0, 2, 1)
    dtpb = np.asarray(inputs["dt_proj_b"], np.float32)[:, own].reshape(L, 4, 128)
    dtpb = dtpb.transpose(0, 2, 1)
    A = -np.exp(np.asarray(inputs["A_log"], np.float32))[:, own, :]  # [L, DH, N]
    # asc[l, p, 16*kt + n] = A[l, kt*128 + p, n]
    asc = A.reshape(L, 4, 128, N).transpose(0, 2, 1, 3).reshape(L, 128, 64)
    dval = np.asarray(inputs["D_skip"], np.float32)[:, own]
    dscdiag = np.zeros((L, 4, 128, 128), np.float32)
    for l in range(L):
        for kt in range(4):
            dscdiag[l, kt, ii, ii] = dval[l, kt * 128:(kt + 1) * 128]
    lnrow = np.asarray(inputs["ln_w"], np.float32).reshape(L, 1, DM)
    lnbc = np.asarray(inputs["ln_b"], np.float32).reshape(L, 4, 128)
    lnbc = lnbc.transpose(0, 2, 1)

    f8 = lambda a: np.ascontiguousarray(a).astype(ml_dtypes.float8_e4m3)
    return dict(
        xt=bf(xt), w1t=bf(w1t), cb=f32(cb), ident=ident,
        identb=bf(np.eye(128, dtype=np.float32)), nwrow=bf(nwrow),
        nbc=f32(nbc),
        wint=bf(wint), wotr=f8(wotr), xpt=bf(xpt), dtpt=bf(dtpt),
        cwdiag=bf(cwdiag), cb1d=f32(cb1d), dtpb=f32(dtpb), asc=f32(asc),
        dscdiag=bf(dscdiag), lnrow=bf(lnrow), lnbc=f32(lnbc),
    )


def kernel(trace=False, **inputs):
    if "nc" not in _CACHE:
        _CACHE["nc"] = _build_nc()
    nc = _CACHE["nc"]
    in_maps = [_prep_core_inputs(c, inputs) for c in range(NC_CORES)]
    res = run_bass_kernel_spmd(nc, in_maps, list(range(NC_CORES)), trace=trace)
    out = np.stack([
        np.repeat(np.asarray(res.results[2 * b]["out"], np.float32),
                  STRIDE, axis=0)[:T]
        for b in range(B)])
    _CACHE["last_result"] = res
    return out


# revision 54
# speedup vs baseline: 146.6278x; 1.0011x over previous
"""Trainium2 Bass kernel for the ConvBranch (Mamba-style) model.

Sharding: 8 cores = 4 batches x 2 DI-halves.
  core c -> batch b = c//2, half m = c%2 (owns DI channels [m*512,(m+1)*512)).
Dense matmuls (in_proj/conv/x_proj/out_proj) are replicated within a pair;
the selective-scan trio is sharded by DI-half; gated y halves are exchanged
with chunked AllGathers per layer (overlapped with the scan).

Scan layout (n-quad fused): per (kt, q) process 4 state indices in one
[128, 4, TS+1] tile; a zero separator column between n-segments resets the
scan state, so one tensor_tensor_scan covers 4 independent recurrences.
dBu / tmp are single quad TT ops with the B/C rows broadcast via stride-0
access patterns. Scans and tmp run on DVE (HW GPSIMD cannot execute
TensorScalarPtr or touch PSUM); most dBu quads run on GpSimd, produced
ahead of the scan chain. dA exps on Act; xi*D folds into the y-PSUM via a
diagonal matmul.

The gated-y exchange is fp8(e4m3): two AllGathers per layer ([kt0,kt1],
[kt2,kt3]), the first hidden behind the second half of the scan. out_proj
consumes the fp8 payload directly with DoubleRow fp8 matmuls (fp8 wotr).

Activation tables are loaded manually; two set switches per layer
(SILU block, then LNEXP block).

Output is [TS, DM] per core; the x4 repeat-interleave upsample happens on
the host.
"""

import sys

sys.path.insert(0, "/opt/trn_rl_repo")

from contextlib import ExitStack

import numpy as np
import ml_dtypes

import concourse.bass as bass
import concourse.bacc as bacc
import concourse.tile as tile
from concourse import mybir
from concourse.bass_utils import run_bass_kernel_spmd
from concourse.tile_rust import add_dep_helper

F32 = mybir.dt.float32
BF16 = mybir.dt.bfloat16
FP8 = mybir.dt.float8e4
NPBF16 = ml_dtypes.bfloat16
AF = mybir.ActivationFunctionType
OP = mybir.AluOpType

B, T, F = 4, 2048, 128
DM, L, STRIDE, KF = 512, 4, 4, 2
N, DC, E = 16, 4, 2
DI = E * DM            # 1024
R = (DM + 15) // 16    # 32
K = KF * STRIDE        # 8
TS = T // STRIDE       # 512
TSP = TS + 1           # +1 separator col for n-fused scans
EPS = 1e-5
DH = DI // 2           # 512 channels per core half
NC_CORES = 8
GROUPS = [[0, 1], [2, 3], [4, 5], [6, 7]]
NQ = 4                 # n-quad group size

# act_info.json set ids (gen3): 6 = ln+exp(+square/copy), 10 = gelu, 18 = silu
SET_LNEXP, SET_GELU, SET_SILU = 6, 10, 18

# dBu quad-TT placement (Pool cannot run scans or stt on HW; TT runs at
# 0.42 efficiency there). dBu only needs dtu+bcrep so Pool can produce it
# ahead of the scan chain; tmp stays on DVE (it feeds PE directly).
CC_CHUNKS = 2          # y-exchange chunks per layer (kt tiles per chunk = 2)

_CACHE = {}
_DEBUG = False


def _emit(ctx, tc, ins, out, dbgs=None):
    nc = tc.nc

    def dbg(name, ap):
        if dbgs is None:
            return
        t = nc.dram_tensor("dbg_" + name, list(ap.shape), ap.dtype,
                           kind="ExternalOutput")
        nc.sync.dma_start(out=t[...], in_=ap)
        dbgs.append("dbg_" + name)

    # Manual activation-table management (same-engine ordering-only edges).
    _tbl = {"load": None, "since": []}

    def load_table(set_id):
        inst = nc.scalar.add_instruction(
            mybir.InstLoadActFuncSet(
                name=nc.get_next_instruction_name(),
                act_func_set_id=set_id, ins=[], outs=[]))
        for p in _tbl["since"]:
            add_dep_helper(inst.ins, p, sync=False, reason="act-table order")
        if _tbl["load"] is not None:
            add_dep_helper(inst.ins, _tbl["load"], sync=False,
                           reason="act-table order")
        _tbl["since"] = []
        _tbl["load"] = inst.ins

    def tact(res):
        if _tbl["load"] is not None:
            add_dep_helper(res.ins, _tbl["load"], sync=False,
                           reason="act-table order")
        _tbl["since"].append(res.ins)
        return res

    consts = ctx.enter_context(tc.tile_pool(name="consts", bufs=1))
    wpool1 = ctx.enter_context(tc.tile_pool(name="wpool1", bufs=1))
    work = ctx.enter_context(tc.tile_pool(name="work", bufs=1))
    scana = ctx.enter_context(tc.tile_pool(name="scana", bufs=3))
    scanh = ctx.enter_context(tc.tile_pool(name="scanh", bufs=3))
    scanb = ctx.enter_context(tc.tile_pool(name="scanb", bufs=4))
    scant = ctx.enter_context(tc.tile_pool(name="scant", bufs=2))
    pA = ctx.enter_context(tc.tile_pool(name="pA", bufs=4, space="PSUM"))
    pB = ctx.enter_context(tc.tile_pool(name="pB", bufs=2, space="PSUM"))
    py = ctx.enter_context(tc.tile_pool(name="py", bufs=2, space="PSUM"))
    dram = ctx.enter_context(tc.tile_pool(name="dram", bufs=1, space="DRAM"))

    # ---- persistent constants ----
    xt_sb = wpool1.tile([128, T + K - 1], BF16, name="wotr")
    nc.sync.dma_start(out=xt_sb, in_=ins["xt"][:, :])
    w1t_sb = wpool1.tile([128, K, DM], BF16, name="cwd")
    nc.sync.dma_start(out=w1t_sb, in_=ins["w1t"].rearrange("k f m -> f k m"))
    cb_sb = consts.tile([128, 4], F32)
    nc.sync.dma_start(out=cb_sb, in_=ins["cb"][:, :])
    ident_sb = consts.tile([128, 128], F32)
    nc.sync.dma_start(out=ident_sb, in_=ins["ident"][:, :])
    identb_sb = consts.tile([128, 128], BF16)
    nc.sync.dma_start(out=identb_sb, in_=ins["identb"][:, :])
    nwrow_sb = consts.tile([1, DM], BF16)
    nc.sync.dma_start(out=nwrow_sb, in_=ins["nwrow"][:, :])
    nbc_sb = consts.tile([128, 4], F32)
    nc.sync.dma_start(out=nbc_sb, in_=ins["nbc"][:, :])
    ones128_sb = consts.tile([128, 1], BF16)
    nc.vector.memset(ones128_sb, 1.0)
    eps1_sb = consts.tile([1, 1], F32)
    nc.vector.memset(eps1_sb, EPS)
    onesf_sb = consts.tile([128, 1], F32)
    nc.vector.memset(onesf_sb, 1.0)

    # B/C broadcast target: [128, 2N, TSP]; separator cols zeroed once.
    bcrep = consts.tile([128, 2 * N, TSP], BF16, name="bcrep")
    nc.vector.memset(bcrep[:, :, TS:TSP], 0.0)

    # residual stream h: 4 persistent fp32 tiles [128, TS]
    h = [consts.tile([128, TS], F32, name=f"h{kt}") for kt in range(4)]

    # ---- front conv + gelu ----
    load_table(SET_GELU)
    for mt in range(4):
        ps = pA.tile([128, TS], F32, name="ps")
        for k in range(K):
            nc.tensor.matmul(
                ps,
                w1t_sb[:, k, mt * 128:(mt + 1) * 128],
                xt_sb[:, k:k + T:STRIDE],
                start=(k == 0),
                stop=(k == K - 1),
            )
        tact(nc.scalar.activation(h[mt], ps, AF.Gelu, bias=cb_sb[:, mt:mt + 1]))
    load_table(SET_LNEXP)

    def load_weights(l):
        w = {}

        def wt(name, shape, dt, src, pool=wpool1):
            t = pool.tile(shape, dt, name=name)
            nc.sync.dma_start(out=t, in_=src)
            w[name] = t

        wt("wint", [128, 4, DI + DH], BF16,
           ins["wint"][l].rearrange("(kt p) e -> p kt e", p=128))
        wt("wotr", [128, 8, DM], FP8,
           ins["wotr"][l].rearrange("(kd p) o -> p kd o", p=128))
        wt("xpt", [128, 8, R + 2 * N], BF16,
           ins["xpt"][l].rearrange("(kd p) e -> p kd e", p=128))
        wt("dtpt", [32, DH], BF16, ins["dtpt"][l])
        wt("cwd", [128, 8, DC, 128], BF16,
           ins["cwdiag"][l].rearrange("e k p q -> p e k q"))
        wt("cb1", [128, 8], F32, ins["cb1d"][l])
        wt("dtpb", [128, 4], F32, ins["dtpb"][l])
        wt("asc", [128, 64], F32, ins["asc"][l])
        wt("dscd", [128, 4, 128], BF16,
           ins["dscdiag"][l].rearrange("k p q -> p k q"))
        wt("lnrow", [1, DM], BF16, ins["lnrow"][l])
        wt("lnbc", [128, 4], F32, ins["lnbc"][l])
        return w

    def layernorm(lnrow, lnbc, out_dtype, name, outq=None):
        """LN over the feature (partition) dim of h; affine folded into the
        rank-1 PE broadcasts: hn = (h*A + lb) - Bc, A = lw*rstd, Bc = lw*c.
        If outq is given, write kt slices into outq[:, kt, :]."""
        st_m = pA.tile([128, TS], F32, name="ps")
        st_q = pA.tile([128, TS], F32, name="ps")
        for kt in range(4):
            hbt = work.tile([128, TS], BF16, name=f"hb{kt % 2}")
            nc.gpsimd.tensor_copy(out=hbt, in_=h[kt])
            sq = work.tile([128, TS], BF16, name="sq")
            nc.scalar.activation(sq, h[kt], AF.Square)
            nc.tensor.matmul(st_m[0:1, :], ones128_sb, hbt,
                             start=(kt == 0), stop=(kt == 3))
            nc.tensor.matmul(st_q[0:1, :], ones128_sb, sq,
                             start=(kt == 0), stop=(kt == 3))
        ms = work.tile([1, 2 * TS], F32, name="ms")
        nc.vector.tensor_scalar(ms[:, 0:TS], st_m[0:1, :], 1.0 / DM, None, OP.mult)
        mu2 = work.tile([1, TS], F32, name="mu2")
        nc.vector.tensor_tensor(out=mu2, in0=ms[:, 0:TS], in1=ms[:, 0:TS],
                                op=OP.mult)
        var = work.tile([1, TS], F32, name="var")
        nc.vector.scalar_tensor_tensor(var, st_q[0:1, :], 1.0 / DM, mu2,
                                       OP.mult, OP.subtract)
        # rstd = exp(-0.5 * ln(var + eps)); c = mu * rstd   (packed [1, 2*TS])
        rc = work.tile([1, 2 * TS], F32, name="rc")
        lnv = work.tile([1, TS], F32, name="lnv")
        tact(nc.scalar.activation(lnv, var, AF.Ln, bias=eps1_sb[:, 0:1]))
        tact(nc.scalar.activation(rc[:, 0:TS], lnv, AF.Exp, scale=-0.5))
        nc.vector.tensor_tensor(out=rc[:, TS:2 * TS], in0=ms[:, 0:TS],
                                in1=rc[:, 0:TS], op=OP.mult)
        rcb = work.tile([1, 2 * TS], BF16, name="rcb")
        nc.scalar.copy(rcb, rc)
        outs = []
        for kt in range(4):
            sl = slice(kt * 128, (kt + 1) * 128)
            pa = pB.tile([128, TS], F32, name="pln")
            nc.tensor.matmul(pa, lnrow[0:1, sl], rcb[:, 0:TS],
                             start=True, stop=True)
            pbc = pB.tile([128, TS], F32, name="pln")
            nc.tensor.matmul(pbc, lnrow[0:1, sl], rcb[:, TS:2 * TS],
                             start=True, stop=True)
            t1 = work.tile([128, TS], F32, name="lnt1")
            nc.vector.tensor_tensor(out=t1, in0=h[kt], in1=pa, op=OP.mult)
            o = outq[:, kt, :] if outq is not None else None
            if o is None:
                ot = work.tile([128, TS], out_dtype, name=f"{name}{kt}")
                o = ot
            nc.vector.scalar_tensor_tensor(o, t1, lnbc[:, kt:kt + 1], pbc,
                                           OP.add, OP.subtract)
            outs.append(o)
        return outs

    # zero the causal pads of the conv input tiles once; the per-layer copy
    # only writes cols [DC-1:).
    xi_pad = [work.tile([128, DC - 1 + TS], BF16, name=f"xipad{et}")
              for et in range(8)]
    for et in range(8):
        nc.vector.memset(xi_pad[et][:, 0:DC - 1], 0.0)

    # persistent quad tiles
    hnq = consts.tile([128, 4, TS], BF16, name="hnq")
    xiq = consts.tile([128, 4, TS], BF16, name="xiq")   # silu(conv), own half
    zq = consts.tile([128, 4, TS], BF16, name="zq")     # silu(z)
    wq = consts.tile([128, 4, TSP], BF16, name="wq")    # softplus dt
    dtuq = consts.tile([128, 4, TSP], BF16, name="dtuq")
    nc.vector.memset(wq[:, :, TS:TSP], 0.0)
    nc.vector.memset(dtuq[:, :, TS:TSP], 0.0)

    scan_q_idx = [0]

    wcur = load_weights(0)
    for l in range(L):
        wnext = load_weights(l + 1) if l + 1 < L else None
        wint_sb, wotr_sb, xpt_sb = wcur["wint"], wcur["wotr"], wcur["xpt"]
        dtpt_sb, cwd_sb, cb1_sb = wcur["dtpt"], wcur["cwd"], wcur["cb1"]
        dtpb_sb, asc_sb, dscd_sb = wcur["dtpb"], wcur["asc"], wcur["dscd"]

        # ---- LN ----
        hn = layernorm(wcur["lnrow"], wcur["lnbc"], BF16, "hn", outq=hnq)
        if l == 0:
            dbg("hn0", hnq[:, 0, :])

        # ---- in_proj xi tiles 0..7 (padded for conv) + z tiles ----
        for et in range(8):
            ps = pA.tile([128, TS], F32, name="ps")
            for kt in range(4):
                nc.tensor.matmul(ps, wint_sb[:, kt, et * 128:(et + 1) * 128],
                                 hnq[:, kt, :], start=(kt == 0), stop=(kt == 3))
            if et % 2 == 0:
                nc.scalar.copy(xi_pad[et][:, DC - 1:DC - 1 + TS], ps)
            else:
                nc.vector.tensor_copy(out=xi_pad[et][:, DC - 1:DC - 1 + TS],
                                      in_=ps)

        load_table(SET_SILU)

        # ---- causal depthwise conv1d + silu; x_proj accumulates per et ----
        # x_proj split: B/C rows [R:R+2N] in one PSUM group (finishes first,
        # feeds the broadcast DMA chain), dt rows [0:R] in another.
        psbc = pA.tile([2 * N, TS], F32, name="ps")
        psdt = pB.tile([R, TS], F32, name="pln")
        xits = []
        for et in range(8):
            psc = pA.tile([128, TS], F32, name="ps")
            for k in range(DC):
                nc.tensor.matmul(psc, cwd_sb[:, et, k, :],
                                 xi_pad[et][:, k:k + TS],
                                 start=(k == 0), stop=(k == DC - 1))
            if et < 4:
                xit = xiq[:, et, :]
            else:
                xio = work.tile([128, TS], BF16, name=f"xio{et % 2}")
                xit = xio
            tact(nc.scalar.activation(xit, psc, AF.Silu,
                                      bias=cb1_sb[:, et:et + 1]))
            nc.tensor.matmul(psbc, xpt_sb[:, et, R:R + 2 * N], xit,
                             start=(et == 0), stop=(et == 7))
            xits.append(xit)
        for et in range(8):
            nc.tensor.matmul(psdt, xpt_sb[:, et, 0:R], xits[et],
                             start=(et == 0), stop=(et == 7))
        if l == 0:
            dbg("xi0", xiq[:, 0, :])
        for zt in range(4):
            et = 8 + zt
            ps = pA.tile([128, TS], F32, name="ps")
            for kt in range(4):
                nc.tensor.matmul(ps, wint_sb[:, kt, et * 128:(et + 1) * 128],
                                 hnq[:, kt, :], start=(kt == 0), stop=(kt == 3))
            tact(nc.scalar.activation(zq[:, zt, :], ps, AF.Silu))
        if l == 0:
            dbg("zs0", zq[:, 0, :])

        xbc = work.tile([2 * N, TS], BF16, name="xbc")
        nc.vector.tensor_copy(out=xbc, in_=psbc)
        xdbl = work.tile([R, TS], BF16, name="xdbl")
        nc.vector.tensor_copy(out=xdbl, in_=psdt)
        if l == 0:
            dbg("xdbl", xdbl)

        # ---- stage B/C rows to DRAM, broadcast to all partitions ----
        bcst = dram.tile([2 * N, TS], BF16, name="bcst")
        nc.sync.dma_start(out=bcst, in_=xbc)
        for q in range(4):
            for half in range(2):
                r0 = half * N + NQ * q
                nc.sync.dma_start(
                    out=bcrep[:, r0:r0 + NQ, 0:TS],
                    in_=bcst[r0:r0 + NQ, :].unsqueeze(0).to_broadcast(
                        [128, NQ, TS]))
        if l == 0:
            dbg("brep", bcrep[:, 0, 0:TS])
            dbg("crep", bcrep[:, N, 0:TS])

        # ---- dt: softplus(dt_proj @ dt_raw + b) = ln(1 + exp(.)) ----
        load_table(SET_LNEXP)
        for kt in range(4):
            psd = pA.tile([128, TS], F32, name="ps")
            nc.tensor.matmul(psd, dtpt_sb[:, kt * 128:(kt + 1) * 128],
                             xdbl[:, :], start=True, stop=True)
            edt = work.tile([128, TS], BF16, name="edt")
            tact(nc.scalar.activation(edt, psd, AF.Exp, bias=dtpb_sb[:, kt:kt + 1]))
            tact(nc.scalar.activation(wq[:, kt, 0:TS], edt, AF.Ln,
                                      bias=onesf_sb[:, 0:1]))
            nc.vector.tensor_tensor(out=dtuq[:, kt, 0:TS], in0=wq[:, kt, 0:TS],
                                    in1=xiq[:, kt, :], op=OP.mult)
        if l == 0:
            dbg("w0", wq[:, 0, 0:TS])
            dbg("dtu0", dtuq[:, 0, 0:TS])

        # ---- scan (n-quad fused) + gating + chunked y-exchange ----
        ygq = work.tile([128, 4, TS], FP8, name="ygq")
        pso = None
        for kt in range(4):
            pyt = py.tile([128, TS], F32, name="py")
            # xi * D_skip folded into the PSUM accumulation via a diag matmul
            nc.tensor.matmul(pyt, dscd_sb[:, kt, :], xiq[:, kt, :],
                             start=True, stop=False)
            # produce all dBu quads for this kt up front (mostly on Pool,
            # running ahead of the DVE scan chain)
            dbus = []
            for qg in range(N // NQ):
                n0 = NQ * qg
                dBuq = scanb.tile([128, NQ, TSP], BF16, name="dBuq")
                on_dve = (qg == 0 and kt == 0) or (kt == 3 and qg >= 2)
                teng = nc.vector if on_dve else nc.gpsimd
                teng.tensor_tensor(
                    out=dBuq,
                    in0=dtuq[:, kt, :].unsqueeze(1).to_broadcast([128, NQ, TSP]),
                    in1=bcrep[:, n0:n0 + NQ, :], op=OP.mult)
                dbus.append(dBuq)
            for qg in range(N // NQ):
                n0 = NQ * qg
                dAq = scana.tile([128, NQ, TSP], BF16, name="dAq")
                nc.gpsimd.memset(dAq[:, :, TS:TSP], 0.0)
                for j in range(NQ):
                    col = kt * 16 + n0 + j
                    tact(nc.scalar.activation(dAq[:, j, 0:TS], wq[:, kt, 0:TS],
                                              AF.Exp,
                                              scale=asc_sb[:, col:col + 1]))
                hsq = scanh.tile([128, NQ, TSP], BF16, name="hsq")
                nc.vector.tensor_tensor_scan(
                    hsq[...].rearrange("p q t -> p (q t)"),
                    dAq[...].rearrange("p q t -> p (q t)"),
                    dbus[qg][...].rearrange("p q t -> p (q t)"),
                    0.0, OP.mult, OP.add)
                tmpq = scant.tile([128, NQ, TS], BF16, name="tmpq")
                nc.vector.tensor_tensor(
                    out=tmpq, in0=hsq[:, :, 0:TS],
                    in1=bcrep[:, N + n0:N + n0 + NQ, 0:TS], op=OP.mult)
                for j in range(NQ):
                    nc.tensor.matmul(pyt, identb_sb, tmpq[:, j, :],
                                     start=False,
                                     stop=(qg == N // NQ - 1 and j == NQ - 1))
                if l == 0 and kt == 0 and qg == 0:
                    dbg("dA00", dAq[:, 0, 0:TS])
                    dbg("dBu00", dbus[0][:, 0, 0:TS])
                    dbg("hs00", hsq[:, 0, 0:TS])
            # gating: yg = (y + xi * D) * silu(z)  (z pre-silu'd in zq)
            nc.vector.tensor_tensor(out=ygq[:, kt, :], in0=zq[:, kt, :],
                                    in1=pyt, op=OP.mult)
            if l == 0 and kt == 0:
                dbg("g10", ygq[:, 0, :])

            if kt % 2 == 1:
                c = kt // 2
                # exchange this chunk of gated y
                ccin = dram.tile([2 * 128, TS], FP8, name=f"ccin{c}")
                ccr = ccin.rearrange("(k p) t -> p k t", p=128)
                for j in range(2):
                    nc.sync.dma_start(out=ccr[:, j, :],
                                      in_=ygq[:, c * 2 + j, :])
                ccout = dram.tile([2 * 256, TS], FP8, name=f"ccout{c}")
                nc.gpsimd.collective_compute(
                    "AllGather", OP.bypass, replica_groups=GROUPS,
                    ins=[ccin[:, :]], outs=[ccout[:, :]],
                )
                ygf8 = work.tile([128, 4, TS], FP8, name=f"yg8{c}")
                ccv = ccout.rearrange("(kd p) t -> p kd t", p=128)
                for kd in range(4):
                    nc.sync.dma_start(out=ygf8[:, kd, :], in_=ccv[:, kd, :])
                if pso is None:
                    pso = [pA.tile([128, TS], F32, name="ps"),
                           pA.tile([128, TS], F32, name="ps"),
                           pB.tile([128, TS], F32, name="pln"),
                           pB.tile([128, TS], F32, name="pln")]
                for b in range(2):
                    for mt in range(4):
                        nc.tensor.matmul(
                            pso[mt],
                            wotr_sb[:, c * 4 + 2 * b:c * 4 + 2 * b + 2,
                                    mt * 128:(mt + 1) * 128],
                            ygf8[:, 2 * b:2 * b + 2, :],
                            start=(c == 0 and b == 0),
                            stop=(c == CC_CHUNKS - 1 and b == 1),
                            perf_mode=mybir.MatmulPerfMode.DoubleRow)

        # ---- residual ----
        for mt in range(4):
            nc.vector.tensor_tensor(out=h[mt], in0=h[mt], in1=pso[mt], op=OP.add)
        if l == 0:
            dbg("hl0", h[0])

        wcur = wnext

    # ---- final LN ----
    hnf = layernorm(nwrow_sb, nbc_sb, F32, "hnf")

    # ---- transpose + store ([TS, DM]; upsample happens on the host) ----
    for ct in range(4):
        hT = work.tile([128, DM], F32, name=f"hT{ct}")
        for kt in range(4):
            pt = pA.tile([128, 128], F32, name="ps")
            nc.tensor.transpose(pt, hnf[kt][:, ct * 128:(ct + 1) * 128], ident_sb)
            if kt % 2 == 0:
                nc.vector.tensor_copy(out=hT[:, kt * 128:(kt + 1) * 128], in_=pt)
            else:
                nc.scalar.copy(hT[:, kt * 128:(kt + 1) * 128], pt)
        nc.sync.dma_start(out=out[ct * 128:(ct + 1) * 128, :], in_=hT)


def _build_nc():
    nc = bacc.Bacc("TRN2", num_devices=NC_CORES)
    ins = {}

    def din(name, shape, dt):
        ins[name] = nc.dram_tensor(name, list(shape), dt, kind="ExternalInput")

    din("xt", (128, T + K - 1), BF16)
    din("w1t", (K, 128, DM), BF16)
    din("cb", (128, 4), F32)
    din("ident", (128, 128), F32)
    din("identb", (128, 128), BF16)
    din("nwrow", (1, DM), BF16)
    din("nbc", (128, 4), F32)
    din("wint", (L, DM, DI + DH), BF16)
    din("wotr", (L, DI, DM), FP8)
    din("xpt", (L, DI, R + 2 * N), BF16)
    din("dtpt", (L, R, DH), BF16)
    din("cwdiag", (L, 8, DC, 128, 128), BF16)
    din("cb1d", (L, 128, 8), F32)
    din("dtpb", (L, 128, 4), F32)
    din("asc", (L, 128, 64), F32)
    din("dscdiag", (L, 4, 128, 128), BF16)
    din("lnrow", (L, 1, DM), BF16)
    din("lnbc", (L, 128, 4), F32)
    out = nc.dram_tensor("out", [TS, DM], F32, kind="ExternalOutput")

    dbgs = [] if _DEBUG else None
    with ExitStack() as ctx:
        tc = ctx.enter_context(tile.TileContext(nc))
        _emit(ctx, tc, ins, out, dbgs)
    nc.compile()
    _CACHE["dbgs"] = dbgs
    return nc


def _prep_core_inputs(c, inputs):
    b, m = c // 2, c % 2
    bf = lambda a: np.ascontiguousarray(a).astype(NPBF16)
    f32 = lambda a: np.ascontiguousarray(a).astype(np.float32)

    x = np.asarray(inputs["x"], np.float32)
    xt = np.zeros((128, T + K - 1), np.float32)
    xt[:, K - 1:] = x[b].T
    w1t = np.asarray(inputs["conv_w"], np.float32).transpose(2, 1, 0)  # [K,F,DM]
    cb = np.asarray(inputs["conv_b"], np.float32).reshape(4, 128).T
    ident = np.eye(128, dtype=np.float32)
    nwrow = np.asarray(inputs["norm_w"], np.float32).reshape(1, DM)
    nbc = np.asarray(inputs["norm_b"], np.float32).reshape(4, 128).T

    # per-core DI channel permutation: own half first
    own = np.arange(m * DH, (m + 1) * DH)
    oth = np.arange((1 - m) * DH, (2 - m) * DH)
    perm = np.concatenate([own, oth])

    in_w = np.asarray(inputs["in_proj_w"], np.float32)    # [L, 2*DI, DM]
    wint = np.empty((L, DM, DI + DH), np.float32)
    for l in range(L):
        wtp = in_w[l].T                                   # [DM, 2*DI]
        wint[l, :, :DI] = wtp[:, perm]                    # xi, permuted
        wint[l, :, DI:] = wtp[:, DI + own]                # z own half
    # out_proj rows in chunk-arrival order (unpermuted channels):
    # chunk c: [h0 ch 256c..256c+256, h1 ch 512+256c..512+256c+256]
    wot = np.asarray(inputs["out_proj_w"], np.float32).transpose(0, 2, 1)  # [L,DI,DM]
    KT_PER_CC = 4 // CC_CHUNKS
    row_order = []
    for cc in range(CC_CHUNKS):
        w0 = cc * KT_PER_CC * 128
        row_order.extend(range(w0, w0 + KT_PER_CC * 128))
        row_order.extend(range(DH + w0, DH + w0 + KT_PER_CC * 128))
    wotr = wot[:, row_order, :]
    xpt = np.asarray(inputs["x_proj_w"], np.float32).transpose(0, 2, 1)[:, perm, :]
    dtpt = np.asarray(inputs["dt_proj_w"], np.float32).transpose(0, 2, 1)[:, :, own]
    cw1d = np.asarray(inputs["conv1d_w"], np.float32)[:, perm, :]
    cwdiag = np.zeros((L, 8, DC, 128, 128), np.float32)
    ii = np.arange(128)
    for l in range(L):
        for et in range(8):
            for k in range(DC):
                cwdiag[l, et, k, ii, ii] = cw1d[l, et * 128:(et + 1) * 128, k]
    cb1d = np.asarray(inputs["conv1d_b"], np.float32)[:, perm].reshape(L, 8, 128)
    cb1d = cb1d.transpose(0, 2, 1)
    dtpb = np.asarray(inputs["dt_proj_b"], np.float32)[:, own].reshape(L, 4, 128)
    dtpb = dtpb.transpose(0, 2, 1)
    A = -np.exp(np.asarray(inputs["A_log"], np.float32))[:, own, :]  # [L, DH, N]
    # asc[l, p, 16*kt + n] = A[l, kt*128 + p, n]
    asc = A.reshape(L, 4, 128, N).transpose(0, 2, 1, 3).reshape(L, 128, 64)
    dval = np.asarray(inputs["D_skip"], np.float32)[:, own]
    dscdiag = np.zeros((L, 4, 128, 128), np.float32)
    for l in range(L):
        for kt in range(4):
            dscdiag[l, kt, ii, ii] = dval[l, kt * 128:(kt + 1) * 128]
    lnrow = np.asarray(inputs["ln_w"], np.float32).reshape(L, 1, DM)
    lnbc = np.asarray(inputs["ln_b"], np.float32).reshape(L, 4, 128)
    lnbc = lnbc.transpose(0, 2, 1)

    f8 = lambda a: np.ascontiguousarray(a).astype(ml_dtypes.float8_e4m3)
    return dict(
        xt=bf(xt), w1t=bf(w1t), cb=f32(cb), ident=ident,
        identb=bf(np.eye(128, dtype=np.float32)), nwrow=bf(nwrow),
        nbc=f32(nbc),
        wint=bf(wint), wotr=f8(wotr), xpt=bf(xpt), dtpt=bf(dtpt),
        cwdiag=bf(cwdiag), cb1d=f32(cb1d), dtpb=f32(dtpb), asc=f32(asc),
        dscdiag=bf(dscdiag), lnrow=bf(lnrow), lnbc=f32(lnbc),
    )


def kernel(trace=False, **inputs):
    if "nc" not in _CACHE:
        _CACHE["nc"] = _build_nc()
    nc = _CACHE["nc"]
    in_maps = [_prep_core_inputs(c, inputs) for c in range(NC_CORES)]
    res = run_bass_kernel_spmd(nc, in_maps, list(range(NC_CORES)), trace=trace)
    out = np.stack([
        np.repeat(np.asarray(res.results[2 * b]["out"], np.float32),
                  STRIDE, axis=0)[:T]
        for b in range(B)])
    _CACHE["last_result"] = res
    return out


# revision 91
# speedup vs baseline: 158.3555x; 1.0800x over previous
"""Trainium2 Bass kernel for the ConvBranch (Mamba-style) model.

Sharding: 8 cores = 4 batches x 2 DI-halves.
  core c -> batch b = c//2, half m = c%2 (owns DI channels [m*512,(m+1)*512)).
Dense matmuls (in_proj/conv/x_proj/out_proj) are replicated within a pair;
the selective-scan trio is sharded by DI-half; gated y halves are exchanged
with chunked AllGathers per layer (overlapped with the scan).

Scan layout (n-quad fused): per (kt, q) process 4 state indices in one
[128, 4, TS+1] tile; a zero separator column between n-segments resets the
scan state, so one tensor_tensor_scan covers 4 independent recurrences.
dBu / tmp are single quad TT ops with the B/C rows broadcast via stride-0
access patterns. Scans and tmp run on DVE (HW GPSIMD cannot execute
TensorScalarPtr or touch PSUM); most dBu quads run on GpSimd, produced
ahead of the scan chain. dA exps on Act; xi*D folds into the y-PSUM via a
diagonal matmul.

The gated-y exchange is fp8(e4m3): two AllGathers per layer ([kt0,kt1],
[kt2,kt3]), the first hidden behind the second half of the scan. out_proj
consumes the fp8 payload directly with DoubleRow fp8 matmuls (fp8 wotr).

Activation tables are loaded manually; two set switches per layer
(SILU block, then LNEXP block).

Output is [TS, DM] per core; the x4 repeat-interleave upsample happens on
the host.
"""

import sys

sys.path.insert(0, "/opt/trn_rl_repo")

from contextlib import ExitStack

import numpy as np
import ml_dtypes

import concourse.bass as bass
import concourse.bacc as bacc
import bass_rust
import concourse.tile as tile
from concourse import mybir
from concourse.bass_utils import run_bass_kernel_spmd
from concourse.tile_rust import add_dep_helper

F32 = mybir.dt.float32
BF16 = mybir.dt.bfloat16
FP8 = mybir.dt.float8e4
NPBF16 = ml_dtypes.bfloat16
AF = mybir.ActivationFunctionType
OP = mybir.AluOpType

B, T, F = 4, 2048, 128
DM, L, STRIDE, KF = 512, 4, 4, 2
N, DC, E = 16, 4, 2
DI = E * DM            # 1024
R = (DM + 15) // 16    # 32
K = KF * STRIDE        # 8
TS = T // STRIDE       # 512
TSP = TS + 1           # +1 separator col for n-fused scans
EPS = 1e-5
DH = DI // 2           # 512 channels per core half
NC_CORES = 8
GROUPS = [[0, 1], [2, 3], [4, 5], [6, 7]]
NQ = 4                 # n-quad group size

# act_info.json set ids (gen3): 6 = ln+exp(+square/copy), 10 = gelu, 18 = silu
SET_LNEXP, SET_GELU, SET_SILU = 6, 10, 18

# dBu quad-TT placement (Pool cannot run scans or stt on HW; TT runs at
# 0.42 efficiency there). dBu only needs dtu+bcrep so Pool can produce it
# ahead of the scan chain; tmp stays on DVE (it feeds PE directly).
CC_CHUNKS = 2          # y-exchange chunks per layer (kt tiles per chunk = 2)

_CACHE = {}
_DEBUG = False


def _emit(ctx, tc, ins, out, dbgs=None):
    nc = tc.nc

    def dbg(name, ap):
        if dbgs is None:
            return
        t = nc.dram_tensor("dbg_" + name, list(ap.shape), ap.dtype,
                           kind="ExternalOutput")
        nc.sync.dma_start(out=t[...], in_=ap)
        dbgs.append("dbg_" + name)

    # Manual activation-table management (same-engine ordering-only edges).
    _tbl = {"load": None, "since": []}

    def load_table(set_id):
        inst = nc.scalar.add_instruction(
            mybir.InstLoadActFuncSet(
                name=nc.get_next_instruction_name(),
                act_func_set_id=set_id, ins=[], outs=[]))
        for p in _tbl["since"]:
            add_dep_helper(inst.ins, p, sync=False, reason="act-table order")
        if _tbl["load"] is not None:
            add_dep_helper(inst.ins, _tbl["load"], sync=False,
                           reason="act-table order")
        _tbl["since"] = []
        _tbl["load"] = inst.ins

    def tact(res):
        if _tbl["load"] is not None:
            add_dep_helper(res.ins, _tbl["load"], sync=False,
                           reason="act-table order")
        _tbl["since"].append(res.ins)
        return res

    consts = ctx.enter_context(tc.tile_pool(name="consts", bufs=1))
    wpool1 = ctx.enter_context(tc.tile_pool(name="wpool1", bufs=1))
    work = ctx.enter_context(tc.tile_pool(name="work", bufs=1))
    scana = ctx.enter_context(tc.tile_pool(name="scana", bufs=5))
    scanh = ctx.enter_context(tc.tile_pool(name="scanh", bufs=3))
    scanb = ctx.enter_context(tc.tile_pool(name="scanb", bufs=4))
    scant = ctx.enter_context(tc.tile_pool(name="scant", bufs=2))
    pA = ctx.enter_context(tc.tile_pool(name="pA", bufs=4, space="PSUM"))
    pB = ctx.enter_context(tc.tile_pool(name="pB", bufs=2, space="PSUM"))
    py = ctx.enter_context(tc.tile_pool(name="py", bufs=2, space="PSUM"))
    dram = ctx.enter_context(tc.tile_pool(name="dram", bufs=1, space="DRAM"))

    # ---- persistent constants ----
    xt_sb = wpool1.tile([128, T + K - 1], BF16, name="wotr")
    nc.sync.dma_start(out=xt_sb, in_=ins["xt"][:, :])
    w1t_sb = wpool1.tile([128, K, DM], BF16, name="cwd")
    nc.sync.dma_start(out=w1t_sb, in_=ins["w1t"].rearrange("k f m -> f k m"))
    cb_sb = consts.tile([128, 4], F32)
    nc.sync.dma_start(out=cb_sb, in_=ins["cb"][:, :])
    ident_sb = consts.tile([128, 128], F32)
    nc.sync.dma_start(out=ident_sb, in_=ins["ident"][:, :])
    identb_sb = consts.tile([128, 128], BF16)
    nc.sync.dma_start(out=identb_sb, in_=ins["identb"][:, :])
    nwrow_sb = consts.tile([1, DM], BF16)
    nc.sync.dma_start(out=nwrow_sb, in_=ins["nwrow"][:, :])
    nbc_sb = consts.tile([128, 4], F32)
    nc.sync.dma_start(out=nbc_sb, in_=ins["nbc"][:, :])
    ones128_sb = consts.tile([128, 1], BF16)
    nc.vector.memset(ones128_sb, 1.0)
    eps1_sb = consts.tile([1, 1], F32)
    nc.vector.memset(eps1_sb, EPS)
    onesf_sb = consts.tile([128, 1], F32)
    nc.vector.memset(onesf_sb, 1.0)

    # B/C broadcast target: [128, 2N, TSP]; separator cols zeroed once.
    bcrep = consts.tile([128, 2 * N, TSP], BF16, name="bcrep")
    nc.vector.memset(bcrep[:, :, TS:TSP], 0.0)

    # residual stream h: 4 persistent fp32 tiles [128, TS]
    h = [consts.tile([128, TS], F32, name=f"h{kt}") for kt in range(4)]

    # ---- front conv + gelu ----
    load_table(SET_GELU)
    for mt in range(4):
        ps = pA.tile([128, TS], F32, name="ps")
        for k in range(K):
            nc.tensor.matmul(
                ps,
                w1t_sb[:, k, mt * 128:(mt + 1) * 128],
                xt_sb[:, k:k + T:STRIDE],
                start=(k == 0),
                stop=(k == K - 1),
            )
        tact(nc.scalar.activation(h[mt], ps, AF.Gelu, bias=cb_sb[:, mt:mt + 1]))
    load_table(SET_LNEXP)

    def load_weights(l):
        w = {}

        def wt(name, shape, dt, src, pool=wpool1):
            t = pool.tile(shape, dt, name=name)
            nc.sync.dma_start(out=t, in_=src)
            w[name] = t

        wt("wint", [128, 4, DI + DH], BF16,
           ins["wint"][l].rearrange("(kt p) e -> p kt e", p=128))
        wt("wotr", [128, 8, DM], FP8,
           ins["wotr"][l].rearrange("(kd p) o -> p kd o", p=128))
        wt("xpt", [128, 8, R + 2 * N], BF16,
           ins["xpt"][l].rearrange("(kd p) e -> p kd e", p=128))
        wt("dtpt", [32, DH], BF16, ins["dtpt"][l])
        wt("cwd", [128, 8, DC, 128], BF16,
           ins["cwdiag"][l].rearrange("e k p q -> p e k q"))
        wt("wmisc", [128, 80], F32, ins["wmisc"][l])
        w["cb1"] = w["wmisc"][:, 0:8]
        w["dtpb"] = w["wmisc"][:, 8:12]
        w["asc"] = w["wmisc"][:, 12:76]
        w["lnbc"] = w["wmisc"][:, 76:80]
        wt("dscd", [128, 4, 128], BF16,
           ins["dscdiag"][l].rearrange("k p q -> p k q"))
        wt("lnrow", [1, DM], BF16, ins["lnrow"][l])
        return w

    def layernorm(lnrow, lnbc, out_dtype, name, outq=None):
        """LN over the feature (partition) dim of h; affine folded into the
        rank-1 PE broadcasts: hn = (h*A + lb) - Bc, A = lw*rstd, Bc = lw*c.
        If outq is given, write kt slices into outq[:, kt, :]."""
        st_m = pA.tile([128, TS], F32, name="ps")
        st_q = pA.tile([128, TS], F32, name="ps")
        for kt in range(4):
            hbt = work.tile([128, TS], BF16, name=f"hb{kt % 2}")
            nc.gpsimd.tensor_copy(out=hbt, in_=h[kt])
            sq = work.tile([128, TS], BF16, name="sq")
            nc.scalar.activation(sq, h[kt], AF.Square)
            nc.tensor.matmul(st_m[0:1, :], ones128_sb, hbt,
                             start=(kt == 0), stop=(kt == 3))
            nc.tensor.matmul(st_q[0:1, :], ones128_sb, sq,
                             start=(kt == 0), stop=(kt == 3))
        ms = work.tile([1, 2 * TS], F32, name="ms")
        nc.vector.tensor_scalar(ms[:, 0:TS], st_m[0:1, :], 1.0 / DM, None, OP.mult)
        mu2 = work.tile([1, TS], F32, name="mu2")
        nc.vector.tensor_tensor(out=mu2, in0=ms[:, 0:TS], in1=ms[:, 0:TS],
                                op=OP.mult)
        var = work.tile([1, TS], F32, name="var")
        nc.vector.scalar_tensor_tensor(var, st_q[0:1, :], 1.0 / DM, mu2,
                                       OP.mult, OP.subtract)
        # rstd = exp(-0.5 * ln(var + eps)); c = mu * rstd   (packed [1, 2*TS])
        rc = work.tile([1, 2 * TS], F32, name="rc")
        lnv = work.tile([1, TS], F32, name="lnv")
        tact(nc.scalar.activation(lnv, var, AF.Ln, bias=eps1_sb[:, 0:1]))
        tact(nc.scalar.activation(rc[:, 0:TS], lnv, AF.Exp, scale=-0.5))
        nc.vector.tensor_tensor(out=rc[:, TS:2 * TS], in0=ms[:, 0:TS],
                                in1=rc[:, 0:TS], op=OP.mult)
        rcb = work.tile([1, 2 * TS], BF16, name="rcb")
        nc.scalar.copy(rcb, rc)
        outs = []
        for kt in range(4):
            sl = slice(kt * 128, (kt + 1) * 128)
            pa = pB.tile([128, TS], F32, name="pln")
            nc.tensor.matmul(pa, lnrow[0:1, sl], rcb[:, 0:TS],
                             start=True, stop=True)
            pbc = pB.tile([128, TS], F32, name="pln")
            nc.tensor.matmul(pbc, lnrow[0:1, sl], rcb[:, TS:2 * TS],
                             start=True, stop=True)
            t1 = work.tile([128, TS], F32, name="lnt1")
            nc.vector.tensor_tensor(out=t1, in0=h[kt], in1=pa, op=OP.mult)
            o = outq[:, kt, :] if outq is not None else None
            if o is None:
                ot = work.tile([128, TS], out_dtype, name=f"{name}{kt}")
                o = ot
            nc.vector.scalar_tensor_tensor(o, t1, lnbc[:, kt:kt + 1], pbc,
                                           OP.add, OP.subtract)
            outs.append(o)
        return outs

    # zero the causal pads of the conv input tiles once; the per-layer copy
    # only writes cols [DC-1:).
    xi_pad = [work.tile([128, DC - 1 + TS], BF16, name=f"xipad{et}")
              for et in range(8)]
    for et in range(8):
        nc.vector.memset(xi_pad[et][:, 0:DC - 1], 0.0)

    # persistent quad tiles
    hnq = consts.tile([128, 4, TS], BF16, name="hnq")
    xiq = consts.tile([128, 4, TS], BF16, name="xiq")   # silu(conv), own half
    zq = consts.tile([128, 4, TS], BF16, name="zq")     # silu(z)
    wq = consts.tile([128, 4, TSP], BF16, name="wq")    # softplus dt
    dtuq = consts.tile([128, 4, TSP], BF16, name="dtuq")
    nc.vector.memset(wq[:, :, TS:TSP], 0.0)
    nc.vector.memset(dtuq[:, :, TS:TSP], 0.0)

    scan_q_idx = [0]

    wcur = load_weights(0)
    for l in range(L):
        wnext = load_weights(l + 1) if l + 1 < L else None
        wint_sb, wotr_sb, xpt_sb = wcur["wint"], wcur["wotr"], wcur["xpt"]
        dtpt_sb, cwd_sb, cb1_sb = wcur["dtpt"], wcur["cwd"], wcur["cb1"]
        dtpb_sb, asc_sb, dscd_sb = wcur["dtpb"], wcur["asc"], wcur["dscd"]

        # ---- LN ----
        hn = layernorm(wcur["lnrow"], wcur["lnbc"], BF16, "hn", outq=hnq)
        if l == 0:
            dbg("hn0", hnq[:, 0, :])

        # ---- in_proj xi tiles 0..7 (padded for conv) + z tiles ----
        for et in range(8):
            ps = pA.tile([128, TS], F32, name="ps")
            for kt in range(4):
                nc.tensor.matmul(ps, wint_sb[:, kt, et * 128:(et + 1) * 128],
                                 hnq[:, kt, :], start=(kt == 0), stop=(kt == 3))
            if et % 2 == 0:
                nc.scalar.copy(xi_pad[et][:, DC - 1:DC - 1 + TS], ps)
            else:
                nc.vector.tensor_copy(out=xi_pad[et][:, DC - 1:DC - 1 + TS],
                                      in_=ps)

        load_table(SET_SILU)

        # ---- causal depthwise conv1d + silu; x_proj accumulates per et ----
        # x_proj split: B/C rows [R:R+2N] in one PSUM group (finishes first,
        # feeds the broadcast DMA chain), dt rows [0:R] in another.
        psbc = pA.tile([2 * N, TS], F32, name="ps")
        psdt = pB.tile([R, TS], F32, name="pln")
        xits = []
        for et in range(8):
            psc = pA.tile([128, TS], F32, name="ps")
            for k in range(DC):
                nc.tensor.matmul(psc, cwd_sb[:, et, k, :],
                                 xi_pad[et][:, k:k + TS],
                                 start=(k == 0), stop=(k == DC - 1))
            if et < 4:
                xit = xiq[:, et, :]
            else:
                xio = work.tile([128, TS], BF16, name=f"xio{et % 4}")
                xit = xio
            tact(nc.scalar.activation(xit, psc, AF.Silu,
                                      bias=cb1_sb[:, et:et + 1]))
            nc.tensor.matmul(psbc, xpt_sb[:, et, R:R + 2 * N], xit,
                             start=(et == 0), stop=(et == 7))
            xits.append(xit)
        for et in range(8):
            nc.tensor.matmul(psdt, xpt_sb[:, et, 0:R], xits[et],
                             start=(et == 0), stop=(et == 7))
        if l == 0:
            dbg("xi0", xiq[:, 0, :])
        for zt in range(4):
            et = 8 + zt
            ps = pA.tile([128, TS], F32, name="ps")
            for kt in range(4):
                nc.tensor.matmul(ps, wint_sb[:, kt, et * 128:(et + 1) * 128],
                                 hnq[:, kt, :], start=(kt == 0), stop=(kt == 3))
            tact(nc.scalar.activation(zq[:, zt, :], ps, AF.Silu))
        if l == 0:
            dbg("zs0", zq[:, 0, :])

        xbc = work.tile([2 * N, TS], BF16, name="xbc")
        nc.vector.tensor_copy(out=xbc, in_=psbc)
        xdbl = work.tile([R, TS], BF16, name="xdbl")
        nc.vector.tensor_copy(out=xdbl, in_=psdt)
        if l == 0:
            dbg("xdbl", xdbl)

        # ---- stage B/C rows to DRAM, broadcast to all partitions ----
        bcst = dram.tile([2 * N, TS], BF16, name="bcst")
        nc.sync.dma_start(out=bcst, in_=xbc)
        for q in range(4):
            for half in range(2):
                r0 = half * N + NQ * q
                nc.sync.dma_start(
                    out=bcrep[:, r0:r0 + NQ, 0:TS],
                    in_=bcst[r0:r0 + NQ, :].unsqueeze(0).to_broadcast(
                        [128, NQ, TS]))
        if l == 0:
            dbg("brep", bcrep[:, 0, 0:TS])
            dbg("crep", bcrep[:, N, 0:TS])

        # ---- dt: softplus(dt_proj @ dt_raw + b) = ln(1 + exp(.)) ----
        load_table(SET_LNEXP)
        for kt in range(4):
            psd = pA.tile([128, TS], F32, name="ps")
            nc.tensor.matmul(psd, dtpt_sb[:, kt * 128:(kt + 1) * 128],
                             xdbl[:, :], start=True, stop=True)
            edt = work.tile([128, TS], BF16, name="edt")
            tact(nc.scalar.activation(edt, psd, AF.Exp, bias=dtpb_sb[:, kt:kt + 1]))
            tact(nc.scalar.activation(wq[:, kt, 0:TS], edt, AF.Ln,
                                      bias=onesf_sb[:, 0:1]))
            nc.vector.tensor_tensor(out=dtuq[:, kt, 0:TS], in0=wq[:, kt, 0:TS],
                                    in1=xiq[:, kt, :], op=OP.mult)
        if l == 0:
            dbg("w0", wq[:, 0, 0:TS])
            dbg("dtu0", dtuq[:, 0, 0:TS])

        # ---- scan (n-quad fused) + gating + chunked y-exchange ----
        ygq = work.tile([128, 4, TS], FP8, name="ygq")
        pso = None
        for kt in range(4):
            pyt = py.tile([128, TS], F32, name="py")
            # xi * D_skip folded into the PSUM accumulation via a diag matmul
            nc.tensor.matmul(pyt, dscd_sb[:, kt, :], xiq[:, kt, :],
                             start=True, stop=False)
            # produce all dBu quads for this kt up front (mostly on Pool,
            # running ahead of the DVE scan chain)
            dbus = []
            for qg in range(N // NQ):
                n0 = NQ * qg
                dBuq = scanb.tile([128, NQ, TSP], BF16, name="dBuq")
                on_dve = (kt == 0 and qg <= 1) or (qg == 0) or (qg == 3)
                teng = nc.vector if on_dve else nc.gpsimd
                teng.tensor_tensor(
                    out=dBuq,
                    in0=dtuq[:, kt, :].unsqueeze(1).to_broadcast([128, NQ, TSP]),
                    in1=bcrep[:, n0:n0 + NQ, :], op=OP.mult)
                dbus.append(dBuq)
            for qg in range(N // NQ):
                n0 = NQ * qg
                dAq = scana.tile([128, NQ, TSP], BF16, name="dAq")
                nc.gpsimd.memset(dAq[:, :, TS:TSP], 0.0)
                for j in range(NQ):
                    col = kt * 16 + n0 + j
                    tact(nc.scalar.activation(dAq[:, j, 0:TS], wq[:, kt, 0:TS],
                                              AF.Exp,
                                              scale=asc_sb[:, col:col + 1]))
                hsq = scanh.tile([128, NQ, TSP], BF16, name="hsq")
                nc.vector.tensor_tensor_scan(
                    hsq[...].rearrange("p q t -> p (q t)"),
                    dAq[...].rearrange("p q t -> p (q t)"),
                    dbus[qg][...].rearrange("p q t -> p (q t)"),
                    0.0, OP.mult, OP.add)
                tmpq = scant.tile([128, NQ, TS], BF16, name="tmpq")
                nc.vector.tensor_tensor(
                    out=tmpq, in0=hsq[:, :, 0:TS],
                    in1=bcrep[:, N + n0:N + n0 + NQ, 0:TS], op=OP.mult)
                for j in range(NQ):
                    nc.tensor.matmul(pyt, identb_sb, tmpq[:, j, :],
                                     start=False,
                                     stop=(qg == N // NQ - 1 and j == NQ - 1))
                if l == 0 and kt == 0 and qg == 0:
                    dbg("dA00", dAq[:, 0, 0:TS])
                    dbg("dBu00", dbus[0][:, 0, 0:TS])
                    dbg("hs00", hsq[:, 0, 0:TS])
            # gating: yg = (y + xi * D) * silu(z)  (z pre-silu'd in zq)
            nc.vector.tensor_tensor(out=ygq[:, kt, :], in0=zq[:, kt, :],
                                    in1=pyt, op=OP.mult)
            if l == 0 and kt == 0:
                dbg("g10", ygq[:, 0, :])

            if kt % 2 == 1:
                c = kt // 2
                # exchange this chunk of gated y
                ccin = dram.tile([2 * 128, TS], FP8, name=f"ccin{c}")
                ccr = ccin.rearrange("(k p) t -> p k t", p=128)
                for j in range(2):
                    nc.sync.dma_start(out=ccr[:, j, :],
                                      in_=ygq[:, c * 2 + j, :])
                ccout = dram.tile([2 * 256, TS], FP8, name=f"ccout{c}")
                nc.gpsimd.collective_compute(
                    "AllGather", OP.bypass, replica_groups=GROUPS,
                    ins=[ccin[:, :]], outs=[ccout[:, :]],
                )
                ygf8 = work.tile([128, 4, TS], FP8, name=f"yg8{c}")
                ccv = ccout.rearrange("(kd p) t -> p kd t", p=128)
                for b in range(2):
                    nc.sync.dma_start(out=ygf8[:, 2 * b:2 * b + 2, :],
                                      in_=ccv[:, 2 * b:2 * b + 2, :])
                if pso is None:
                    pso = [pA.tile([128, TS], F32, name="ps"),
                           pA.tile([128, TS], F32, name="ps"),
                           pB.tile([128, TS], F32, name="pln"),
                           pB.tile([128, TS], F32, name="pln")]
                for b in range(2):
                    for mt in range(4):
                        nc.tensor.matmul(
                            pso[mt],
                            wotr_sb[:, c * 4 + 2 * b:c * 4 + 2 * b + 2,
                                    mt * 128:(mt + 1) * 128],
                            ygf8[:, 2 * b:2 * b + 2, :],
                            start=(c == 0 and b == 0),
                            stop=(c == CC_CHUNKS - 1 and b == 1),
                            perf_mode=mybir.MatmulPerfMode.DoubleRow)

        # ---- residual ----
        for mt in range(4):
            nc.vector.tensor_tensor(out=h[mt], in0=h[mt], in1=pso[mt], op=OP.add)
        if l == 0:
            dbg("hl0", h[0])

        wcur = wnext

    # ---- final LN ----
    hnf = layernorm(nwrow_sb, nbc_sb, F32, "hnf")

    # ---- transpose + store ([TS, DM]; upsample happens on the host) ----
    for ct in range(4):
        hT = work.tile([128, DM], F32, name=f"hT{ct}")
        for kt in range(4):
            pt = pA.tile([128, 128], F32, name="ps")
            nc.tensor.transpose(pt, hnf[kt][:, ct * 128:(ct + 1) * 128], ident_sb)
            if kt % 2 == 0:
                nc.vector.tensor_copy(out=hT[:, kt * 128:(kt + 1) * 128], in_=pt)
            else:
                nc.scalar.copy(hT[:, kt * 128:(kt + 1) * 128], pt)
        nc.sync.dma_start(out=out[ct * 128:(ct + 1) * 128, :], in_=hT)


def _build_nc():
    nc = bacc.Bacc("TRN2", num_devices=NC_CORES)
    ins = {}

    def din(name, shape, dt):
        ins[name] = nc.dram_tensor(name, list(shape), dt, kind="ExternalInput")

    din("xt", (128, T + K - 1), BF16)
    din("w1t", (K, 128, DM), BF16)
    din("cb", (128, 4), F32)
    din("ident", (128, 128), F32)
    din("identb", (128, 128), BF16)
    din("nwrow", (1, DM), BF16)
    din("nbc", (128, 4), F32)
    din("wint", (L, DM, DI + DH), BF16)
    din("wotr", (L, DI, DM), FP8)
    din("xpt", (L, DI, R + 2 * N), BF16)
    din("dtpt", (L, R, DH), BF16)
    din("cwdiag", (L, 8, DC, 128, 128), BF16)
    din("wmisc", (L, 128, 80), F32)
    din("dscdiag", (L, 4, 128, 128), BF16)
    din("lnrow", (L, 1, DM), BF16)
    out = nc.dram_tensor("out", [TS, DM], F32, kind="ExternalOutput")

    dbgs = [] if _DEBUG else None
    with ExitStack() as ctx:
        tc = ctx.enter_context(tile.TileContext(nc))
        _emit(ctx, tc, ins, out, dbgs)
    nc.compile()
    _CACHE["dbgs"] = dbgs
    return nc


def _prep_core_inputs(c, inputs):
    b, m = c // 2, c % 2
    bf = lambda a: np.ascontiguousarray(a).astype(NPBF16)
    f32 = lambda a: np.ascontiguousarray(a).astype(np.float32)

    x = np.asarray(inputs["x"], np.float32)
    xt = np.zeros((128, T + K - 1), np.float32)
    xt[:, K - 1:] = x[b].T
    w1t = np.asarray(inputs["conv_w"], np.float32).transpose(2, 1, 0)  # [K,F,DM]
    cb = np.asarray(inputs["conv_b"], np.float32).reshape(4, 128).T
    ident = np.eye(128, dtype=np.float32)
    nwrow = np.asarray(inputs["norm_w"], np.float32).reshape(1, DM)
    nbc = np.asarray(inputs["norm_b"], np.float32).reshape(4, 128).T

    # per-core DI channel permutation: own half first
    own = np.arange(m * DH, (m + 1) * DH)
    oth = np.arange((1 - m) * DH, (2 - m) * DH)
    perm = np.concatenate([own, oth])

    in_w = np.asarray(inputs["in_proj_w"], np.float32)    # [L, 2*DI, DM]
    wint = np.empty((L, DM, DI + DH), np.float32)
    for l in range(L):
        wtp = in_w[l].T                                   # [DM, 2*DI]
        wint[l, :, :DI] = wtp[:, perm]                    # xi, permuted
        wint[l, :, DI:] = wtp[:, DI + own]                # z own half
    # out_proj rows in chunk-arrival order (unpermuted channels):
    # chunk c: [h0 ch 256c..256c+256, h1 ch 512+256c..512+256c+256]
    wot = np.asarray(inputs["out_proj_w"], np.float32).transpose(0, 2, 1)  # [L,DI,DM]
    KT_PER_CC = 4 // CC_CHUNKS
    row_order = []
    for cc in range(CC_CHUNKS):
        w0 = cc * KT_PER_CC * 128
        row_order.extend(range(w0, w0 + KT_PER_CC * 128))
        row_order.extend(range(DH + w0, DH + w0 + KT_PER_CC * 128))
    wotr = wot[:, row_order, :]
    xpt = np.asarray(inputs["x_proj_w"], np.float32).transpose(0, 2, 1)[:, perm, :]
    dtpt = np.asarray(inputs["dt_proj_w"], np.float32).transpose(0, 2, 1)[:, :, own]
    cw1d = np.asarray(inputs["conv1d_w"], np.float32)[:, perm, :]
    cwdiag = np.zeros((L, 8, DC, 128, 128), np.float32)
    ii = np.arange(128)
    for l in range(L):
        for et in range(8):
            for k in range(DC):
                cwdiag[l, et, k, ii, ii] = cw1d[l, et * 128:(et + 1) * 128, k]
    cb1d = np.asarray(inputs["conv1d_b"], np.float32)[:, perm].reshape(L, 8, 128)
    cb1d = cb1d.transpose(0, 2, 1)
    dtpb = np.asarray(inputs["dt_proj_b"], np.float32)[:, own].reshape(L, 4, 128)
    dtpb = dtpb.transpose(0, 2, 1)
    A = -np.exp(np.asarray(inputs["A_log"], np.float32))[:, own, :]  # [L, DH, N]
    # asc[l, p, 16*kt + n] = A[l, kt*128 + p, n]
    asc = A.reshape(L, 4, 128, N).transpose(0, 2, 1, 3).reshape(L, 128, 64)
    dval = np.asarray(inputs["D_skip"], np.float32)[:, own]
    dscdiag = np.zeros((L, 4, 128, 128), np.float32)
    for l in range(L):
        for kt in range(4):
            dscdiag[l, kt, ii, ii] = dval[l, kt * 128:(kt + 1) * 128]
    lnrow = np.asarray(inputs["ln_w"], np.float32).reshape(L, 1, DM)
    lnbc = np.asarray(inputs["ln_b"], np.float32).reshape(L, 4, 128)
    lnbc = lnbc.transpose(0, 2, 1)

    f8 = lambda a: np.ascontiguousarray(a).astype(ml_dtypes.float8_e4m3)
    return dict(
        xt=bf(xt), w1t=bf(w1t), cb=f32(cb), ident=ident,
        identb=bf(np.eye(128, dtype=np.float32)), nwrow=bf(nwrow),
        nbc=f32(nbc),
        wint=bf(wint), wotr=f8(wotr), xpt=bf(xpt), dtpt=bf(dtpt),
        cwdiag=bf(cwdiag),
        wmisc=f32(np.concatenate(
            [cb1d, dtpb, asc, lnbc], axis=2)),
        dscdiag=bf(dscdiag), lnrow=bf(lnrow),
    )


def kernel(trace=False, **inputs):
    if "nc" not in _CACHE:
        _CACHE["nc"] = _build_nc()
    nc = _CACHE["nc"]
    in_maps = [_prep_core_inputs(c, inputs) for c in range(NC_CORES)]
    res = run_bass_kernel_spmd(nc, in_maps, list(range(NC_CORES)), trace=trace)
    out = np.stack([
        np.repeat(np.asarray(res.results[2 * b]["out"], np.float32),
                  STRIDE, axis=0)[:T]
        for b in range(B)])
    _CACHE["last_result"] = res
    return out
